# revision 14
# baseline (speedup 1.0000x reference)
"""DistMaps kernel for Trainium2 (Bass), SPMD over 8 NeuronCores.

Problem: out[b, 0, z, y, x] = 1.0 if min_p ((z-pz)^2 + (y-py)^2 + (x-px)^2) <= 25
over the 24 points p of batch b, else 0.0.  (The x input is only used for its
shape.)

Strategy (v2)
-------------
Shard the volume over D (96 = 8 cores x 12 z-slices); each core gets only the
points whose +-5 z-window intersects its slab (<= 12, padded with far-away
dummies), so no collective is needed.

Per z-slice the plane is a union of disks; row by row a disk is an x-interval:
ind[k, x] = (x-px)^2 <= 25 - (y-py)^2 - (z-pz)^2 for k-rows (point, row offset
j in -4..5).  With onehot[k, y] = (y == floor(py)+j):

  plane[y, x] = sum_k onehot[k, y] * ind_z[k, x]   (TensorE matmul, bf16)
  out = plane > 0                                  (DVE is_gt / ACT Sign -> u8)

v2 changes vs v1 (28.9us):
  * all per-point tables (dx2[k,x], tts[k,z], onehot[k,y]) precomputed on the
    HOST and shipped as two small DMAs (~215 KB) -- removes the 4us serialized
    DVE setup chain from the critical path.
  * rhs indicators built in 4 fused DVE tensor_tensor ops (double-broadcast)
    instead of 24 sliced ops.
  * 6 matmuls instead of 16: per batch one 960-col matmul per zgroup-pair for
    y<128 and ONE 1920-col matmul for the y>=128 remainder (both batches share
    one [64,1920] PSUM via tile_position).
  * thresholds write uint8 (0/1 is exact); the host casts to f32 on unshard.
    4x fewer output bytes -> store drain ~2us instead of ~8us.
  * stores chase each pair threshold on the sync ring (overlapped with the
    rest of the pipeline).
"""

import numpy as np

B = 2
D, H, W = 96, 160, 160
P = 24
J = 10
NCORES = 8
DLOC = D // NCORES   # 12
ZG = 3               # z-slices per matmul group (tile fallback)
NZG = DLOC // ZG     # 4
R2 = 25.0
NPTS_FAST = 12
K = NPTS_FAST * J    # 120 k-rows per batch (fast path)
ZH = DLOC // 2       # 6 z-slices per zgroup-half (fast path)
NWARM = 5
WARMC = 480

_prog_cache = {}


def _build_program_fast(npts):
    import concourse.mybir as mybir
    from concourse import bacc

    assert npts == NPTS_FAST
    f32 = mybir.dt.float32
    f16 = mybir.dt.float16
    u8 = mybir.dt.uint8
    op = mybir.AluOpType
    Act = mybir.ActivationFunctionType

    NW = ZH * W            # 960 cols per zgroup-half
    NWD = DLOC * W         # 1920 cols per batch
    HW2 = NW // 2          # 480: matmul dsts must fit in one 512-f32 PSUM bank

    nc = bacc.Bacc(trn_type="TRN2")

    # rank-space tables, fp16 (exact for the small ints involved):
    #   dxr[k, (b,x)] in 1..160, ttr[k, (b,z)] in 0..160
    tab_d = nc.dram_tensor("tab", [128, 2 * W + 2 * DLOC], f16, kind="ExternalInput")
    oh_d = nc.dram_tensor("oh", [128, 2 * H], f16, kind="ExternalInput")
    # out0: pair stores, pair p = b*2 + h covers z in [h*6, h*6+6), y in [0,128)
    out0_d = nc.dram_tensor("out0", [2 * B, 128, NW], u8, kind="ExternalOutput")
    # out1: y in [128,160): partitions (b, h, yo), cols (zr, x), z = h*6+zr
    out1_d = nc.dram_tensor("out1", [128, NW], u8, kind="ExternalOutput")

    s_in = nc.alloc_semaphore("s_in")
    s_dve = nc.alloc_semaphore("s_dve")
    s_act = nc.alloc_semaphore("s_act")
    s_pe = nc.alloc_semaphore("s_pe")
    s_st = nc.alloc_semaphore("s_st")
    sem_nums = [s.num for s in (s_in, s_dve, s_act, s_pe, s_st)]
    assert max(sem_nums) - min(sem_nums) + 1 == len(sem_nums), sem_nums
    sem_range = range(min(sem_nums), max(sem_nums) + 1)

    tab = nc.alloc_sbuf_tensor("tab_s", [128, 2 * W + 2 * DLOC], f16)
    oh = nc.alloc_sbuf_tensor("oh_s", [128, 2 * H], f16)
    warm_a = nc.alloc_sbuf_tensor("warm_a", [128, 512], f16)
    rhs = [nc.alloc_sbuf_tensor(f"rhs{b}", [128, NWD], f16) for b in range(B)]
    ob0 = [nc.alloc_sbuf_tensor(f"ob0_{p}", [128, NW], u8) for p in range(2 * B)]
    ob1 = nc.alloc_sbuf_tensor("ob1", [128, NW], u8)

    # 3 ps0 slots (2 banks each) + ps1 (2 banks) = 8 banks
    ps0 = [nc.alloc_psum_tensor(f"ps0_{i}", [128, 1024], f32) for i in range(3)]
    ps1 = nc.alloc_psum_tensor("ps1", [128, 1024], f32)

    # ---- SP: input table load (the DVE-critical one) ----
    nc.sync.dma_start(out=tab[:, :], in_=tab_d[:, :], single_packet=True).then_inc(
        s_in, 16
    )

    # ---- ACT ring: onehot load (PE-critical), then dummy activation so the
    # ~1.3us ACT_TABLE_LOAD runs during the input-DMA flight. ----
    nc.scalar.dma_start(out=oh[:, :], in_=oh_d[:, :], single_packet=True).then_inc(
        s_st, 16
    )
    actscratch = nc.alloc_sbuf_tensor("actscratch", [1, 4], f32)
    nc.scalar.activation(out=actscratch[0:1, :], in_=actscratch[0:1, :], func=Act.Sign)

    # ---- PE: continuous HAM warm-up until the first rhs lands ----
    for _ in range(NWARM):
        nc.tensor.matmul(
            out=ps0[0][:, 0:WARMC], lhsT=warm_a[:, 0:128], rhs=warm_a[:, 0:WARMC],
            start=True, stop=True,
        )

    dxr = [tab[:K, b * W : (b + 1) * W] for b in range(B)]
    ttr = [tab[:K, 2 * W + b * DLOC : 2 * W + (b + 1) * DLOC] for b in range(B)]
    c = {"dve": 0, "act": 0, "pe": 0, "st": 0}
    rhs_done = {}

    # ---- DVE: all 4 rhs builds, fp16 in/out (2x DVE rate), fused is_le ----
    nc.vector.wait_ge(s_in, 16)
    for b, h in ((0, 0), (0, 1), (1, 0), (1, 1)):
        nc.vector.tensor_tensor(
            out=rhs[b][:K, h * NW : (h + 1) * NW].rearrange("p (z x) -> p z x", x=W),
            in0=dxr[b].rearrange("p x -> p () x").to_broadcast([K, ZH, W]),
            in1=ttr[b][:, h * ZH : (h + 1) * ZH]
            .rearrange("p z -> p z ()")
            .to_broadcast([K, ZH, W]),
            op=op.is_le,
        ).then_inc(s_dve, 1)
        c["dve"] += 1
        rhs_done[(b, h)] = ("dve", c["dve"])

    oh128 = [oh[:K, b * H : b * H + 128] for b in range(B)]
    oh32 = [oh[:K, b * H + 128 : b * H + 160] for b in range(B)]
    mm_done = {}
    thr_done = {}
    nc.tensor.wait_ge(s_st, 16)   # oh landed

    def wait_pe(token):
        eng, v = token
        nc.tensor.wait_ge(s_dve if eng == "dve" else s_act, v)

    def emit_mm_pair(b, h, slot, extra_wait=None):
        pi = b * 2 + h
        wait_pe(rhs_done[(b, h)])
        if extra_wait is not None:
            wait_pe(extra_wait)
        for q in range(2):
            ins = nc.tensor.matmul(
                out=ps0[slot][:, q * 512 : q * 512 + HW2],
                lhsT=oh128[b],
                rhs=rhs[b][:K, h * NW + q * HW2 : h * NW + (q + 1) * HW2],
                start=True, stop=True,
            )
        ins.then_inc(s_pe, 1)
        c["pe"] += 1
        mm_done[pi] = c["pe"]

    def emit_mm_ps1(b, h):
        # quadrant (b,h) -> partitions b*64 + h*32 + yo, cols (zr, x)
        po = b * 64 + h * 32
        wait_pe(rhs_done[(b, h)])
        for q in range(2):
            ins = nc.tensor.matmul(
                out=ps1[po : po + 32, q * 512 : q * 512 + HW2],
                lhsT=oh32[b],
                rhs=rhs[b][:K, h * NW + q * HW2 : h * NW + (q + 1) * HW2],
                start=True, stop=True,
                tile_position=(0, po),
            )
        ins.then_inc(s_pe, 1)
        c["pe"] += 1
        mm_done[("ps1", b, h)] = c["pe"]

    def emit_thr(pi, E, slot):
        src = ps0[slot][:, :].rearrange("p (q c) -> p q c", c=512)[:, :, 0:HW2]
        dst = ob0[pi][:, :].rearrange("p (q c) -> p q c", c=HW2)
        if E == "dve":
            nc.vector.wait_ge(s_pe, mm_done[pi])
            nc.vector.tensor_scalar(
                out=dst, in0=src, scalar1=0.0, scalar2=None, op0=op.is_gt,
            ).then_inc(s_dve, 1)
            c["dve"] += 1
            thr_done[pi] = ("dve", c["dve"])
        else:
            nc.scalar.wait_ge(s_pe, mm_done[pi])
            nc.scalar.activation(out=dst, in_=src, func=Act.Sign).then_inc(s_act, 1)
            c["act"] += 1
            thr_done[pi] = ("act", c["act"])

    # PE order: pair11 last, ps1 quads before it so the ps1 threshold overlaps
    # the pair11 matmuls; pair11's threshold is split across DVE+ACT halves.
    emit_mm_pair(0, 0, slot=0)
    emit_mm_pair(0, 1, slot=1)
    emit_mm_ps1(0, 0)
    emit_mm_ps1(0, 1)
    emit_thr(0, "act", slot=0)
    emit_thr(1, "dve", slot=1)
    emit_mm_pair(1, 0, slot=2)
    emit_mm_ps1(1, 0)
    emit_mm_ps1(1, 1)
    emit_thr(2, "act", slot=2)
    emit_mm_pair(1, 1, slot=0, extra_wait=thr_done[0])

    # ps1 threshold halves (overlap with pair11 matmuls)
    ps1_all = mm_done[("ps1", 1, 1)]
    nc.vector.wait_ge(s_pe, ps1_all)
    nc.vector.tensor_scalar(
        out=ob1[:, 0:HW2], in0=ps1[:, 0:HW2],
        scalar1=0.0, scalar2=None, op0=op.is_gt,
    ).then_inc(s_dve, 1)
    c["dve"] += 1
    ps1_q0 = c["dve"]
    nc.scalar.wait_ge(s_pe, ps1_all)
    nc.scalar.activation(
        out=ob1[:, HW2:NW], in_=ps1[:, 512 : 512 + HW2], func=Act.Sign
    ).then_inc(s_act, 1)
    c["act"] += 1
    ps1_q1 = c["act"]

    # pair-3 threshold split in halves across both engines (the tail item)
    nc.vector.wait_ge(s_pe, mm_done[3])
    nc.vector.tensor_scalar(
        out=ob0[3][:, 0:HW2], in0=ps0[0][:, 0:HW2],
        scalar1=0.0, scalar2=None, op0=op.is_gt,
    ).then_inc(s_dve, 1)
    c["dve"] += 1
    thr3_q0 = c["dve"]
    nc.scalar.wait_ge(s_pe, mm_done[3])
    nc.scalar.activation(
        out=ob0[3][:, HW2:NW], in_=ps0[0][:, 512 : 512 + HW2], func=Act.Sign
    ).then_inc(s_act, 1)
    c["act"] += 1
    thr3_q1 = c["act"]

    # ---- SP: stores chase thresholds; pair-3 last ----
    for pi in range(3):
        eng, v = thr_done[pi]
        nc.sync.wait_ge(s_dve if eng == "dve" else s_act, v)
        nc.sync.dma_start(out=out0_d[pi, :, :], in_=ob0[pi][:, :]).then_inc(s_st, 16)
        c["st"] += 16
    nc.sync.wait_ge(s_dve, ps1_q0)
    nc.sync.wait_ge(s_act, ps1_q1)
    nc.sync.dma_start(out=out1_d[:, :], in_=ob1[:, :]).then_inc(s_st, 16)
    c["st"] += 16
    nc.sync.wait_ge(s_dve, thr3_q0)
    nc.sync.wait_ge(s_act, thr3_q1)
    nc.sync.dma_start(out=out0_d[3, :, :], in_=ob0[3][:, :]).then_inc(s_st, 16)
    c["st"] += 16

    # ---- GPSIMD: wait for stores + oh-load, reset sems for re-execution ----
    nc.gpsimd.wait_ge(s_st, c["st"] + 16)
    nc.gpsimd.wait_ge(s_in, 16)
    nc.gpsimd.dma_reset(sem_range)
    nc.gpsimd.sem_clear(sem_range)

    nc.finalize()
    return nc


def _build_in_maps_fast(coords: np.ndarray, sel):
    """Host-precomputed rank-space tables per core (fp16, exact small ints).

    dxr[k, x] = rank (1..160) of dx2[k, x] = (x - px)^2 among x' (stable).
    ttr[k, z] = #{x: dx2[k, x] <= tts[k, z]}  (count of in-interval x).
    Then (dx2 <= tts) <=> (dxr <= ttr), exactly, with all values small
    integers that fp16 represents exactly.  dx2/tts are computed in f32 with
    the same operation order as the f32 device path had, so the indicator is
    bit-identical to the previous (verified-exact) kernel.
    """
    jcol = np.arange(-4, 6, dtype=np.float32)
    xs = np.arange(W, dtype=np.float32)
    ys = np.arange(H, dtype=np.float32)

    in_maps = []
    for core in range(NCORES):
        zbase = core * DLOC
        zs = np.arange(zbase, zbase + DLOC, dtype=np.float32)
        tab = np.zeros((128, 2 * W + 2 * DLOC), dtype=np.float16)
        oh = np.zeros((128, 2 * H), dtype=np.float16)
        for b in range(B):
            pb = sel[core][b]
            pts = np.full((NPTS_FAST, 3), 1.0e9, dtype=np.float32)
            pts[: len(pb)] = pb
            pz = np.repeat(pts[:, 0], J)              # (K,)
            py = np.repeat(pts[:, 1], J)
            px = np.repeat(pts[:, 2], J)
            jj = np.tile(jcol, NPTS_FAST)
            yrow = np.floor(py).astype(np.float32) + jj   # (K,)
            dyj = (yrow - py).astype(np.float32) ** 2
            cc = (np.float32(R2) - dyj).astype(np.float32)
            dz2 = ((zs[None, :] - pz[:, None]).astype(np.float32) ** 2).astype(
                np.float32
            )
            tts = (cc[:, None] - dz2).astype(np.float32)          # (K, DLOC)
            dx2 = ((xs[None, :] - px[:, None]).astype(np.float32) ** 2).astype(
                np.float32
            )                                                      # (K, W)
            # rank-space: stable ranks 1..W along x, counts along z
            order = np.argsort(dx2, axis=1, kind="stable")
            dxr = np.empty((K, W), dtype=np.float16)
            np.put_along_axis(
                dxr, order, np.arange(1, W + 1, dtype=np.float16)[None, :], axis=1
            )
            ttr = (dx2[:, None, :] <= tts[:, :, None]).sum(axis=2)  # (K, DLOC)
            tab[:K, b * W : (b + 1) * W] = dxr
            tab[:K, 2 * W + b * DLOC : 2 * W + (b + 1) * DLOC] = ttr.astype(
                np.float16
            )
            oh[:K, b * H : (b + 1) * H] = (ys[None, :] == yrow[:, None]).astype(
                np.float16
            )
        in_maps.append(
            {
                "tab": np.ascontiguousarray(tab),
                "oh": np.ascontiguousarray(oh),
            }
        )
    return in_maps


def _select_points(coords: np.ndarray):
    coords = np.ascontiguousarray(coords, dtype=np.float32)
    assert coords.shape == (B * P, 3)
    sel = []
    maxn = 0
    for core in range(NCORES):
        z0, z1 = core * DLOC, (core + 1) * DLOC
        per_b = []
        for b in range(B):
            cb = coords[b * P : (b + 1) * P]
            m = (cb[:, 0] > z0 - 5.001) & (cb[:, 0] < z1 + 5.001)
            pb = cb[m]
            per_b.append(pb)
            maxn = max(maxn, len(pb))
        sel.append(per_b)
    return sel, maxn


# ---------------------------------------------------------------------------
# Tile-scheduled fallback for NPTS > 12 (unchanged from v1).
# ---------------------------------------------------------------------------

def _build_program_tile(npts):
    from contextlib import ExitStack

    import concourse.mybir as mybir
    import concourse.tile as tile
    from concourse import bacc

    f32 = mybir.dt.float32
    bf16 = mybir.dt.bfloat16
    op = mybir.AluOpType

    K = npts * J                     # k-rows per batch
    NRG = (K + 127) // 128           # row groups per batch
    NB = B * NRG                     # total (b, g) blocks, stacked along free dim
    rgs = [min(128, K - 128 * g) for g in range(NRG)]  # rows per group

    nc = bacc.Bacc(trn_type="TRN2")

    pts_d = nc.dram_tensor("pts", [4, NB, 128], f32, kind="ExternalInput")
    zgb_d = nc.dram_tensor("zgridb", [128, DLOC], f32, kind="ExternalInput")
    xgb_d = nc.dram_tensor("xgridb", [128, W], f32, kind="ExternalInput")
    out_d = nc.dram_tensor("out", [B, H, DLOC, W], f32, kind="ExternalOutput")

    with tile.TileContext(nc) as tc, ExitStack() as ctx:
        const = ctx.enter_context(tc.tile_pool(name="const", bufs=1))
        setup = ctx.enter_context(tc.tile_pool(name="setup", bufs=1))
        rhsp = ctx.enter_context(tc.tile_pool(name="rhsp", bufs=4))
        outp = ctx.enter_context(tc.tile_pool(name="outp", bufs=8))
        psump = ctx.enter_context(tc.tile_pool(name="psump", bufs=4, space="PSUM"))
        warmp = ctx.enter_context(tc.tile_pool(name="warmp", bufs=1, space="PSUM"))

        xgb = const.tile([128, W], f32, name="xgb")
        nc.sync.dma_start(out=xgb[:, :], in_=xgb_d[:, :])
        zgb = const.tile([128, DLOC], f32, name="zgb")
        nc.sync.dma_start(out=zgb[:, :], in_=zgb_d[:, :])
        psb = setup.tile([128, 4 * NB], f32, name="psb")
        nc.sync.dma_start(out=psb[:, :], in_=pts_d[:, :])

        warm_a = const.tile([128, 512], bf16, name="warm_a")
        nc.vector.memset(warm_a[:, :], 0.0)
        warm_ps = warmp.tile([128, 512], f32, name="warm_ps")
        for w in range(14):
            nc.tensor.matmul(
                out=warm_ps[:, :], lhsT=warm_a[:, 0:128], rhs=warm_a[:, :],
                start=True, stop=True,
            )

        pzc = psb[:, 0 * NB : 1 * NB]
        pyc = psb[:, 1 * NB : 2 * NB]
        pxc = psb[:, 2 * NB : 3 * NB]
        jcc = psb[:, 3 * NB : 4 * NB]

        col = setup.tile([128, 4 * NB], f32, name="col")
        yfl = col[:, 0 * NB : 1 * NB]
        ypr = col[:, 1 * NB : 2 * NB]
        dyj = col[:, 2 * NB : 3 * NB]
        cc = col[:, 3 * NB : 4 * NB]
        MAGIC = float(12582912.0)  # 1.5 * 2**23
        nc.vector.tensor_scalar(
            out=yfl, in0=pyc, scalar1=-0.5, scalar2=MAGIC, op0=op.add, op1=op.add
        )
        nc.vector.tensor_scalar(
            out=yfl, in0=yfl, scalar1=MAGIC, scalar2=None, op0=op.subtract
        )
        nc.vector.tensor_tensor(out=ypr, in0=yfl, in1=jcc, op=op.add)
        nc.vector.tensor_tensor(out=dyj, in0=ypr, in1=pyc, op=op.subtract)
        nc.vector.tensor_tensor(out=dyj, in0=dyj, in1=dyj, op=op.mult)
        nc.vector.tensor_scalar(
            out=cc, in0=dyj, scalar1=-1.0, scalar2=R2, op0=op.mult, op1=op.add
        )

        dzt = setup.tile([128, NB * DLOC], f32, name="dzt")
        zgb_v = zgb[:, :].rearrange("p z -> p () z").to_broadcast([128, NB, DLOC])
        nc.vector.tensor_tensor(
            out=dzt[:, :].rearrange("p (blk z) -> p blk z", z=DLOC),
            in0=zgb_v,
            in1=pzc.rearrange("p blk -> p blk ()").to_broadcast([128, NB, DLOC]),
            op=op.subtract,
        )
        nc.vector.tensor_tensor(out=dzt[:, :], in0=dzt[:, :], in1=dzt[:, :], op=op.mult)
        tts = setup.tile([128, NB * DLOC], f32, name="tts")
        nc.vector.tensor_tensor(
            out=tts[:, :].rearrange("p (blk z) -> p blk z", z=DLOC),
            in0=cc.rearrange("p blk -> p blk ()").to_broadcast([128, NB, DLOC]),
            in1=dzt[:, :].rearrange("p (blk z) -> p blk z", z=DLOC),
            op=op.subtract,
        )

        dx2 = setup.tile([128, NB * W], f32, name="dx2")
        xgb_v = xgb[:, :].rearrange("p x -> p () x").to_broadcast([128, NB, W])
        nc.vector.tensor_tensor(
            out=dx2[:, :].rearrange("p (blk x) -> p blk x", x=W),
            in0=xgb_v,
            in1=pxc.rearrange("p blk -> p blk ()").to_broadcast([128, NB, W]),
            op=op.subtract,
        )
        nc.vector.tensor_tensor(out=dx2[:, :], in0=dx2[:, :], in1=dx2[:, :], op=op.mult)

        oh = setup.tile([128, NB * W], bf16, name="oh")
        nc.vector.tensor_tensor(
            out=oh[:, :].rearrange("p (blk x) -> p blk x", x=W),
            in0=xgb_v,
            in1=ypr.rearrange("p blk -> p blk ()").to_broadcast([128, NB, W]),
            op=op.is_equal,
        )

        def blk(b, g):
            return b * NRG + g

        NW = ZG * W  # 480
        ps1big = {
            b: psump.tile([128, NW], f32, name=f"ps1_{b}", tag=f"ps1_{b}", bufs=1)
            for b in range(B)
        }
        ob1big = {b: outp.tile([128, NW], f32, name=f"ob1_{b}", bufs=1) for b in range(B)}

        for b in range(B):
            for zg in range(NZG):
                rhs = []
                for g in range(NRG):
                    rg = rgs[g]
                    rhs_g = rhsp.tile([128, NW], bf16, name=f"rhs{g}", tag=f"rhs{g}")
                    for zi in range(ZG):
                        z = zg * ZG + zi
                        dst = rhs_g[:rg, zi * W : (zi + 1) * W]
                        src = dx2[:rg, blk(b, g) * W : blk(b, g) * W + W]
                        tcol = tts[:rg, blk(b, g) * DLOC + z : blk(b, g) * DLOC + z + 1]
                        if zi == 1:
                            nc.scalar.activation(
                                out=dst, in_=src,
                                func=mybir.ActivationFunctionType.Relu,
                                bias=tcol, scale=-1.0,
                            )
                        else:
                            nc.vector.tensor_scalar(
                                out=dst, in0=src, scalar1=tcol, scalar2=None,
                                op0=op.is_le,
                            )
                    rhs.append(rhs_g)

                ps0 = psump.tile([128, NW], f32, name="ps0", tag="ps0")
                for g in range(NRG):
                    rg = rgs[g]
                    nc.tensor.matmul(
                        out=ps0[:, :],
                        lhsT=oh[:rg, blk(b, g) * W : blk(b, g) * W + 128],
                        rhs=rhs[g][:rg, :],
                        start=(g == 0), stop=(g == NRG - 1),
                    )
                for g in range(NRG):
                    rg = rgs[g]
                    nc.tensor.matmul(
                        out=ps1big[b][32 * zg : 32 * zg + 32, :],
                        lhsT=oh[:rg, blk(b, g) * W + 128 : blk(b, g) * W + 160],
                        rhs=rhs[g][:rg, :],
                        start=(g == 0), stop=(g == NRG - 1),
                        tile_position=(0, 32 * zg),
                    )

                ob0 = outp.tile([128, NW], f32, name="ob0", tag="ob0")
                if zg % 2 == 0:
                    nc.scalar.activation(
                        out=ob0[:, :], in_=ps0[:, :],
                        func=mybir.ActivationFunctionType.Sign,
                    )
                else:
                    nc.vector.tensor_scalar(
                        out=ob0[:, :], in0=ps0[:, :], scalar1=0.0, scalar2=None,
                        op0=op.is_gt,
                    )
                zl = zg * ZG
                nc.sync.dma_start(
                    out=out_d[b, 0:128, zl : zl + ZG, :],
                    in_=ob0[:, :].rearrange("p (z x) -> p z x", z=ZG),
                )

            nc.vector.tensor_scalar(
                out=ob1big[b][:, :], in0=ps1big[b][:, :], scalar1=0.0, scalar2=None,
                op0=op.is_gt,
            )
            for zg in range(NZG):
                zl = zg * ZG
                nc.sync.dma_start(
                    out=out_d[b, 128:160, zl : zl + ZG, :],
                    in_=ob1big[b][32 * zg : 32 * zg + 32, :].rearrange(
                        "p (z x) -> p z x", z=ZG
                    ),
                )

    nc.finalize()
    return nc


def _build_in_maps_tile(coords: np.ndarray, sel, NPTS):
    K = NPTS * J
    NRG = (K + 127) // 128
    NB = B * NRG
    jcol = np.tile(np.arange(-4, 6, dtype=np.float32), NPTS)

    xgridb = np.ascontiguousarray(
        np.broadcast_to(np.arange(W, dtype=np.float32), (128, W))
    )

    in_maps = []
    for core in range(NCORES):
        zbase = core * DLOC
        zgridb = np.ascontiguousarray(
            np.broadcast_to(
                np.arange(zbase, zbase + DLOC, dtype=np.float32), (128, DLOC)
            )
        )
        pts2 = np.empty((4, NB, 128), dtype=np.float32)
        pts2[0:3] = 1.0e9
        pts2[3] = 0.0
        for b in range(B):
            pb = sel[core][b]
            krows = np.repeat(pb, J, axis=0)
            n = krows.shape[0]
            for g in range(NRG):
                lo, hi = g * 128, min((g + 1) * 128, n)
                if lo >= n:
                    break
                bk = b * NRG + g
                pts2[0, bk, 0 : hi - lo] = krows[lo:hi, 0]
                pts2[1, bk, 0 : hi - lo] = krows[lo:hi, 1]
                pts2[2, bk, 0 : hi - lo] = krows[lo:hi, 2]
                pts2[3, bk, 0 : hi - lo] = jcol[lo:hi]
        in_maps.append(
            {
                "pts": np.ascontiguousarray(pts2),
                "zgridb": zgridb,
                "xgridb": xgridb,
            }
        )
    return in_maps


def _build_in_maps(coords: np.ndarray):
    sel, maxn = _select_points(coords)
    if maxn <= NPTS_FAST:
        return NPTS_FAST, _build_in_maps_fast(coords, sel)
    NPTS = ((maxn + 3) // 4) * 4
    return NPTS, _build_in_maps_tile(coords, sel, NPTS)


def _get_program(npts):
    if npts not in _prog_cache:
        if npts <= NPTS_FAST:
            _prog_cache[npts] = _build_program_fast(npts)
        else:
            _prog_cache[npts] = _build_program_tile(npts)
    return _prog_cache[npts]


def kernel(x: np.ndarray, coords: np.ndarray) -> np.ndarray:
    from concourse.bass_utils import run_bass_kernel_spmd

    assert x.shape == (B, 4, D, H, W)
    NPTS, in_maps = _build_in_maps(coords)
    nc = _get_program(NPTS)
    res = run_bass_kernel_spmd(nc, in_maps, list(range(NCORES)))

    full = np.empty((B, 1, D, H, W), dtype=np.float32)
    for core in range(NCORES):
        zsl = slice(core * DLOC, (core + 1) * DLOC)
        r = res.results[core]
        if NPTS <= NPTS_FAST:
            # out0: [4, 128, 960] u8, pair p = b*2+h -> z in [6h,6h+6), y<128
            o0 = r["out0"].reshape(B, 2, 128, ZH, W)
            full[:, 0, zsl, 0:128, :] = (
                o0.transpose(0, 1, 3, 2, 4).reshape(B, DLOC, 128, W)
            )
            # out1: [128, 960] u8, partitions (b, h, yo), cols (zr, x)
            o1 = r["out1"].reshape(B, 2, 32, ZH, W)
            full[:, 0, zsl, 128:160, :] = (
                o1.transpose(0, 1, 3, 2, 4).reshape(B, DLOC, 32, W)
            )
        else:
            o = r["out"]  # [B, H, DLOC, W] f32
            full[:, 0, zsl] = o.transpose(0, 2, 1, 3)
    return full


# revision 21
# speedup vs baseline: 1.0403x; 1.0403x over previous
"""DistMaps kernel for Trainium2 (Bass), SPMD over 8 NeuronCores.

Problem: out[b, 0, z, y, x] = 1.0 if min_p ((z-pz)^2 + (y-py)^2 + (x-px)^2) <= 25
over the 24 points p of batch b, else 0.0.  (The x input is only used for its
shape.)

Strategy (v2)
-------------
Shard the volume over D (96 = 8 cores x 12 z-slices); each core gets only the
points whose +-5 z-window intersects its slab (<= 12, padded with far-away
dummies), so no collective is needed.

Per z-slice the plane is a union of disks; row by row a disk is an x-interval:
ind[k, x] = (x-px)^2 <= 25 - (y-py)^2 - (z-pz)^2 for k-rows (point, row offset
j in -4..5).  With onehot[k, y] = (y == floor(py)+j):

  plane[y, x] = sum_k onehot[k, y] * ind_z[k, x]   (TensorE matmul, bf16)
  out = plane > 0                                  (DVE is_gt / ACT Sign -> u8)

v2 changes vs v1 (28.9us):
  * all per-point tables (dx2[k,x], tts[k,z], onehot[k,y]) precomputed on the
    HOST and shipped as two small DMAs (~215 KB) -- removes the 4us serialized
    DVE setup chain from the critical path.
  * rhs indicators built in 4 fused DVE tensor_tensor ops (double-broadcast)
    instead of 24 sliced ops.
  * 6 matmuls instead of 16: per batch one 960-col matmul per zgroup-pair for
    y<128 and ONE 1920-col matmul for the y>=128 remainder (both batches share
    one [64,1920] PSUM via tile_position).
  * thresholds write uint8 (0/1 is exact); the host casts to f32 on unshard.
    4x fewer output bytes -> store drain ~2us instead of ~8us.
  * stores chase each pair threshold on the sync ring (overlapped with the
    rest of the pipeline).
"""

import numpy as np

B = 2
D, H, W = 96, 160, 160
P = 24
J = 10
NCORES = 8
DLOC = D // NCORES   # 12
ZG = 3               # z-slices per matmul group (tile fallback)
NZG = DLOC // ZG     # 4
R2 = 25.0
NPTS_FAST = 12
K = NPTS_FAST * J    # 120 k-rows per batch (fast path)
ZH = DLOC // 2       # 6 z-slices per zgroup-half (fast path)
NWARM = 6
WARMC = 480
NYC = 120            # compressed-y partitions (<= 120 distinct onehot rows)
USE_POOL_RHS = False

_prog_cache = {}


def _build_program_fast(npts):
    import concourse.mybir as mybir
    from concourse import bacc

    assert npts == NPTS_FAST
    f32 = mybir.dt.float32
    f16 = mybir.dt.float16
    u8 = mybir.dt.uint8
    op = mybir.AluOpType
    Act = mybir.ActivationFunctionType

    NW = ZH * W            # 960 cols per zgroup-half
    NWD = DLOC * W         # 1920 cols per batch
    HW2 = NW // 2          # 480: matmul dsts must fit in one 512-f32 PSUM bank

    nc = bacc.Bacc(trn_type="TRN2")

    # rank-space tables, fp16 (exact small ints); one chunk per batch so the
    # b0 tables are semaphore-visible earlier.
    tab0_d = nc.dram_tensor("tab0", [128, W + DLOC], f16, kind="ExternalInput")
    tab1_d = nc.dram_tensor("tab1", [128, W + DLOC], f16, kind="ExternalInput")
    # compressed onehot: oh[k, b*NYC + yc] = (yrow[k] == Y_active[b][yc])
    oh_d = nc.dram_tensor("oh", [128, 2 * NYC], f16, kind="ExternalInput")
    # out0[pair p=b*2+h, yc, (zr, x)]: z = h*6+zr, y = Y_active[b][yc]
    out0_d = nc.dram_tensor("out0", [2 * B, NYC, NW], u8, kind="ExternalOutput")

    s_in = nc.alloc_semaphore("s_in")
    s_in2 = nc.alloc_semaphore("s_in2")
    s_dve = nc.alloc_semaphore("s_dve")
    s_act = nc.alloc_semaphore("s_act")
    s_pe = nc.alloc_semaphore("s_pe")
    s_st = nc.alloc_semaphore("s_st")
    sem_nums = [s.num for s in (s_in, s_in2, s_dve, s_act, s_pe, s_st)]
    assert max(sem_nums) - min(sem_nums) + 1 == len(sem_nums), sem_nums
    sem_range = range(min(sem_nums), max(sem_nums) + 1)

    tab = [
        nc.alloc_sbuf_tensor(f"tab{b}_s", [128, W + DLOC], f16) for b in range(B)
    ]
    oh = nc.alloc_sbuf_tensor("oh_s", [128, 2 * NYC], f16)
    warm_a = nc.alloc_sbuf_tensor("warm_a", [128, 512], f16)
    rhs = [nc.alloc_sbuf_tensor(f"rhs{b}", [128, NWD], f16) for b in range(B)]
    ob0 = [nc.alloc_sbuf_tensor(f"ob0_{p}", [128, NW], u8) for p in range(2 * B)]

    # 4 ps0 slots (2 banks each) = 8 banks; no slot reuse -> no PSUM waits
    ps0 = [nc.alloc_psum_tensor(f"ps0_{i}", [128, 1024], f32) for i in range(4)]

    # ---- SP: tab0 on the sync ring ----
    nc.sync.dma_start(out=tab[0][:, :], in_=tab0_d[:, :], single_packet=True).then_inc(
        s_in, 16
    )

    # ---- SP ring: tab1 right behind tab0 (b0 tables are the critical ones) ----
    nc.sync.dma_start(out=tab[1][:, :], in_=tab1_d[:, :], single_packet=True).then_inc(
        s_in2, 16
    )

    # ---- ACT ring: onehot load, then dummy activation for the table preload ----
    nc.scalar.dma_start(out=oh[:, :], in_=oh_d[:, :], single_packet=True).then_inc(
        s_st, 16
    )
    actscratch = nc.alloc_sbuf_tensor("actscratch", [1, 4], f32)
    nc.scalar.activation(out=actscratch[0:1, :], in_=actscratch[0:1, :], func=Act.Sign)

    # ---- PE: continuous HAM warm-up until the first rhs lands ----
    for _ in range(NWARM):
        nc.tensor.matmul(
            out=ps0[0][:, 0:WARMC], lhsT=warm_a[:, 0:128], rhs=warm_a[:, 0:WARMC],
            start=True, stop=True,
        )

    dxr = [tab[b][:K, 0:W] for b in range(B)]
    ttr = [tab[b][:K, W : W + DLOC] for b in range(B)]
    c = {"dve": 0, "act": 0, "pe": 0, "st": 0}
    rhs_done = {}

    def emit_rhs(b, h, eng, sem, cnt):
        e = {"dve": nc.vector, "pool": nc.gpsimd}[eng]
        e.tensor_tensor(
            out=rhs[b][:K, h * NW : (h + 1) * NW].rearrange("p (z x) -> p z x", x=W),
            in0=dxr[b].rearrange("p x -> p () x").to_broadcast([K, ZH, W]),
            in1=ttr[b][:, h * ZH : (h + 1) * ZH]
            .rearrange("p z -> p z ()")
            .to_broadcast([K, ZH, W]),
            op=op.is_le,
        ).then_inc(sem, 1)
        c[cnt] += 1
        rhs_done[(b, h)] = (cnt, c[cnt])

    # DVE: (0,0), (0,1), (1,0); Pool: (1,1) in parallel
    nc.vector.wait_ge(s_in, 16)
    emit_rhs(0, 0, "dve", s_dve, "dve")
    emit_rhs(0, 1, "dve", s_dve, "dve")
    nc.vector.wait_ge(s_in2, 16)
    emit_rhs(1, 0, "dve", s_dve, "dve")
    if USE_POOL_RHS:
        # pool rhs signals on s_in2 (tab1's sem, already at 16) -> 17
        nc.gpsimd.wait_ge(s_in2, 16)
        c["in2"] = 16
        emit_rhs(1, 1, "pool", s_in2, "in2")
    else:
        emit_rhs(1, 1, "dve", s_dve, "dve")

    ohc = [oh[:K, b * NYC : (b + 1) * NYC] for b in range(B)]
    mm_done = {}
    thr_done = {}
    nc.tensor.wait_ge(s_st, 16)   # oh landed

    sems = {"dve": s_dve, "act": s_act, "in2": s_in2}

    def emit_mm_pair(b, h, slot):
        pi = b * 2 + h
        eng, v = rhs_done[(b, h)]
        nc.tensor.wait_ge(sems[eng], v)
        for q in range(2):
            ins = nc.tensor.matmul(
                out=ps0[slot][:NYC, q * 512 : q * 512 + HW2],
                lhsT=ohc[b],
                rhs=rhs[b][:K, h * NW + q * HW2 : h * NW + (q + 1) * HW2],
                start=True, stop=True,
            )
        ins.then_inc(s_pe, 1)
        c["pe"] += 1
        mm_done[pi] = c["pe"]

    def emit_thr_half(pi, q, E, slot):
        src = ps0[slot][:NYC, q * 512 : q * 512 + HW2]
        dst = ob0[pi][:NYC, q * HW2 : (q + 1) * HW2]
        if E == "dve":
            nc.vector.wait_ge(s_pe, mm_done[pi])
            nc.vector.tensor_scalar(
                out=dst, in0=src, scalar1=0.0, scalar2=None, op0=op.is_gt,
            ).then_inc(s_dve, 1)
            c["dve"] += 1
            return ("dve", c["dve"])
        else:
            nc.scalar.wait_ge(s_pe, mm_done[pi])
            nc.scalar.activation(out=dst, in_=src, func=Act.Sign).then_inc(s_act, 1)
            c["act"] += 1
            return ("act", c["act"])

    def emit_thr(pi, E, slot):
        src = ps0[slot][:NYC, :].rearrange("p (q c) -> p q c", c=512)[:, :, 0:HW2]
        dst = ob0[pi][:NYC, :].rearrange("p (q c) -> p q c", c=HW2)
        if E == "dve":
            nc.vector.wait_ge(s_pe, mm_done[pi])
            nc.vector.tensor_scalar(
                out=dst, in0=src, scalar1=0.0, scalar2=None, op0=op.is_gt,
            ).then_inc(s_dve, 1)
            c["dve"] += 1
            thr_done[pi] = [("dve", c["dve"])]
        else:
            nc.scalar.wait_ge(s_pe, mm_done[pi])
            nc.scalar.activation(out=dst, in_=src, func=Act.Sign).then_inc(s_act, 1)
            c["act"] += 1
            thr_done[pi] = [("act", c["act"])]

    emit_mm_pair(0, 0, slot=0)
    emit_mm_pair(0, 1, slot=1)
    emit_thr(0, "act", slot=0)
    emit_mm_pair(1, 0, slot=2)
    emit_thr(1, "act", slot=1)
    emit_mm_pair(1, 1, slot=3)
    thr_done[2] = [
        emit_thr_half(2, 0, "dve", 2),
        emit_thr_half(2, 1, "act", 2),
    ]
    thr_done[3] = [
        emit_thr_half(3, 0, "dve", 3),
        emit_thr_half(3, 1, "act", 3),
    ]

    # ---- SP: stores chase thresholds ----
    for pi in range(4):
        for eng, v in thr_done[pi]:
            nc.sync.wait_ge(s_dve if eng == "dve" else s_act, v)
        nc.sync.dma_start(
            out=out0_d[pi, :, :], in_=ob0[pi][:NYC, :]
        ).then_inc(s_st, 16)
        c["st"] += 16

    # ---- GPSIMD: wait for stores + loads, reset sems for re-execution ----
    nc.gpsimd.wait_ge(s_st, c["st"] + 16)
    nc.gpsimd.wait_ge(s_in, 16)
    nc.gpsimd.wait_ge(s_in2, 17 if USE_POOL_RHS else 16)
    nc.gpsimd.dma_reset(sem_range)
    nc.gpsimd.sem_clear(sem_range)

    nc.finalize()
    return nc


_YACT = None  # per-core, per-batch active-y lists from the last fast build


def _build_in_maps_fast(coords: np.ndarray, sel):
    """Host-precomputed rank-space tables per core (fp16, exact small ints).

    dxr[k, x] = rank (1..160) of dx2[k, x] = (x - px)^2 among x' (stable).
    ttr[k, z] = #{x: dx2[k, x] <= tts[k, z]}  (count of in-interval x).
    Then (dx2 <= tts) <=> (dxr <= ttr) exactly.  The onehot is compressed to
    the <= 120 distinct active y-rows per (core, b); _YACT records the row
    mapping for the unshard scatter (all other y-rows are provably zero).
    """
    global _YACT
    jcol = np.arange(-4, 6, dtype=np.float32)
    xs = np.arange(W, dtype=np.float32)

    in_maps = []
    _YACT = []
    for core in range(NCORES):
        zbase = core * DLOC
        zs = np.arange(zbase, zbase + DLOC, dtype=np.float32)
        tabs = []
        oh = np.zeros((128, 2 * NYC), dtype=np.float16)
        yact_core = []
        for b in range(B):
            pb = sel[core][b]
            pts = np.full((NPTS_FAST, 3), 1.0e9, dtype=np.float32)
            pts[: len(pb)] = pb
            pz = np.repeat(pts[:, 0], J)              # (K,)
            py = np.repeat(pts[:, 1], J)
            px = np.repeat(pts[:, 2], J)
            jj = np.tile(jcol, NPTS_FAST)
            yrow = np.floor(py).astype(np.float32) + jj   # (K,)
            dyj = (yrow - py).astype(np.float32) ** 2
            cc = (np.float32(R2) - dyj).astype(np.float32)
            dz2 = ((zs[None, :] - pz[:, None]).astype(np.float32) ** 2).astype(
                np.float32
            )
            tts = (cc[:, None] - dz2).astype(np.float32)          # (K, DLOC)
            dx2 = ((xs[None, :] - px[:, None]).astype(np.float32) ** 2).astype(
                np.float32
            )                                                      # (K, W)
            order = np.argsort(dx2, axis=1, kind="stable")
            dxr = np.empty((K, W), dtype=np.float16)
            np.put_along_axis(
                dxr, order, np.arange(1, W + 1, dtype=np.float16)[None, :], axis=1
            )
            ttr = (dx2[:, None, :] <= tts[:, :, None]).sum(axis=2)  # (K, DLOC)
            tab = np.zeros((128, W + DLOC), dtype=np.float16)
            tab[:K, 0:W] = dxr
            tab[:K, W : W + DLOC] = ttr.astype(np.float16)
            tabs.append(np.ascontiguousarray(tab))
            # compressed onehot over the active y rows
            yint = yrow.astype(np.int64)
            yvals = np.unique(yint[(yint >= 0) & (yint < H)])
            assert len(yvals) <= NYC, len(yvals)
            yact_core.append(yvals)
            oh[:K, b * NYC : b * NYC + len(yvals)] = (
                yvals[None, :] == yrow[:, None]
            ).astype(np.float16)
        _YACT.append(yact_core)
        in_maps.append(
            {
                "tab0": tabs[0],
                "tab1": tabs[1],
                "oh": np.ascontiguousarray(oh),
            }
        )
    return in_maps


def _select_points(coords: np.ndarray):
    coords = np.ascontiguousarray(coords, dtype=np.float32)
    assert coords.shape == (B * P, 3)
    sel = []
    maxn = 0
    for core in range(NCORES):
        z0, z1 = core * DLOC, (core + 1) * DLOC
        per_b = []
        for b in range(B):
            cb = coords[b * P : (b + 1) * P]
            m = (cb[:, 0] > z0 - 5.001) & (cb[:, 0] < z1 + 5.001)
            pb = cb[m]
            per_b.append(pb)
            maxn = max(maxn, len(pb))
        sel.append(per_b)
    return sel, maxn


# ---------------------------------------------------------------------------
# Tile-scheduled fallback for NPTS > 12 (unchanged from v1).
# ---------------------------------------------------------------------------

def _build_program_tile(npts):
    from contextlib import ExitStack

    import concourse.mybir as mybir
    import concourse.tile as tile
    from concourse import bacc

    f32 = mybir.dt.float32
    bf16 = mybir.dt.bfloat16
    op = mybir.AluOpType

    K = npts * J                     # k-rows per batch
    NRG = (K + 127) // 128           # row groups per batch
    NB = B * NRG                     # total (b, g) blocks, stacked along free dim
    rgs = [min(128, K - 128 * g) for g in range(NRG)]  # rows per group

    nc = bacc.Bacc(trn_type="TRN2")

    pts_d = nc.dram_tensor("pts", [4, NB, 128], f32, kind="ExternalInput")
    zgb_d = nc.dram_tensor("zgridb", [128, DLOC], f32, kind="ExternalInput")
    xgb_d = nc.dram_tensor("xgridb", [128, W], f32, kind="ExternalInput")
    out_d = nc.dram_tensor("out", [B, H, DLOC, W], f32, kind="ExternalOutput")

    with tile.TileContext(nc) as tc, ExitStack() as ctx:
        const = ctx.enter_context(tc.tile_pool(name="const", bufs=1))
        setup = ctx.enter_context(tc.tile_pool(name="setup", bufs=1))
        rhsp = ctx.enter_context(tc.tile_pool(name="rhsp", bufs=4))
        outp = ctx.enter_context(tc.tile_pool(name="outp", bufs=8))
        psump = ctx.enter_context(tc.tile_pool(name="psump", bufs=4, space="PSUM"))
        warmp = ctx.enter_context(tc.tile_pool(name="warmp", bufs=1, space="PSUM"))

        xgb = const.tile([128, W], f32, name="xgb")
        nc.sync.dma_start(out=xgb[:, :], in_=xgb_d[:, :])
        zgb = const.tile([128, DLOC], f32, name="zgb")
        nc.sync.dma_start(out=zgb[:, :], in_=zgb_d[:, :])
        psb = setup.tile([128, 4 * NB], f32, name="psb")
        nc.sync.dma_start(out=psb[:, :], in_=pts_d[:, :])

        warm_a = const.tile([128, 512], bf16, name="warm_a")
        nc.vector.memset(warm_a[:, :], 0.0)
        warm_ps = warmp.tile([128, 512], f32, name="warm_ps")
        for w in range(14):
            nc.tensor.matmul(
                out=warm_ps[:, :], lhsT=warm_a[:, 0:128], rhs=warm_a[:, :],
                start=True, stop=True,
            )

        pzc = psb[:, 0 * NB : 1 * NB]
        pyc = psb[:, 1 * NB : 2 * NB]
        pxc = psb[:, 2 * NB : 3 * NB]
        jcc = psb[:, 3 * NB : 4 * NB]

        col = setup.tile([128, 4 * NB], f32, name="col")
        yfl = col[:, 0 * NB : 1 * NB]
        ypr = col[:, 1 * NB : 2 * NB]
        dyj = col[:, 2 * NB : 3 * NB]
        cc = col[:, 3 * NB : 4 * NB]
        MAGIC = float(12582912.0)  # 1.5 * 2**23
        nc.vector.tensor_scalar(
            out=yfl, in0=pyc, scalar1=-0.5, scalar2=MAGIC, op0=op.add, op1=op.add
        )
        nc.vector.tensor_scalar(
            out=yfl, in0=yfl, scalar1=MAGIC, scalar2=None, op0=op.subtract
        )
        nc.vector.tensor_tensor(out=ypr, in0=yfl, in1=jcc, op=op.add)
        nc.vector.tensor_tensor(out=dyj, in0=ypr, in1=pyc, op=op.subtract)
        nc.vector.tensor_tensor(out=dyj, in0=dyj, in1=dyj, op=op.mult)
        nc.vector.tensor_scalar(
            out=cc, in0=dyj, scalar1=-1.0, scalar2=R2, op0=op.mult, op1=op.add
        )

        dzt = setup.tile([128, NB * DLOC], f32, name="dzt")
        zgb_v = zgb[:, :].rearrange("p z -> p () z").to_broadcast([128, NB, DLOC])
        nc.vector.tensor_tensor(
            out=dzt[:, :].rearrange("p (blk z) -> p blk z", z=DLOC),
            in0=zgb_v,
            in1=pzc.rearrange("p blk -> p blk ()").to_broadcast([128, NB, DLOC]),
            op=op.subtract,
        )
        nc.vector.tensor_tensor(out=dzt[:, :], in0=dzt[:, :], in1=dzt[:, :], op=op.mult)
        tts = setup.tile([128, NB * DLOC], f32, name="tts")
        nc.vector.tensor_tensor(
            out=tts[:, :].rearrange("p (blk z) -> p blk z", z=DLOC),
            in0=cc.rearrange("p blk -> p blk ()").to_broadcast([128, NB, DLOC]),
            in1=dzt[:, :].rearrange("p (blk z) -> p blk z", z=DLOC),
            op=op.subtract,
        )

        dx2 = setup.tile([128, NB * W], f32, name="dx2")
        xgb_v = xgb[:, :].rearrange("p x -> p () x").to_broadcast([128, NB, W])
        nc.vector.tensor_tensor(
            out=dx2[:, :].rearrange("p (blk x) -> p blk x", x=W),
            in0=xgb_v,
            in1=pxc.rearrange("p blk -> p blk ()").to_broadcast([128, NB, W]),
            op=op.subtract,
        )
        nc.vector.tensor_tensor(out=dx2[:, :], in0=dx2[:, :], in1=dx2[:, :], op=op.mult)

        oh = setup.tile([128, NB * W], bf16, name="oh")
        nc.vector.tensor_tensor(
            out=oh[:, :].rearrange("p (blk x) -> p blk x", x=W),
            in0=xgb_v,
            in1=ypr.rearrange("p blk -> p blk ()").to_broadcast([128, NB, W]),
            op=op.is_equal,
        )

        def blk(b, g):
            return b * NRG + g

        NW = ZG * W  # 480
        ps1big = {
            b: psump.tile([128, NW], f32, name=f"ps1_{b}", tag=f"ps1_{b}", bufs=1)
            for b in range(B)
        }
        ob1big = {b: outp.tile([128, NW], f32, name=f"ob1_{b}", bufs=1) for b in range(B)}

        for b in range(B):
            for zg in range(NZG):
                rhs = []
                for g in range(NRG):
                    rg = rgs[g]
                    rhs_g = rhsp.tile([128, NW], bf16, name=f"rhs{g}", tag=f"rhs{g}")
                    for zi in range(ZG):
                        z = zg * ZG + zi
                        dst = rhs_g[:rg, zi * W : (zi + 1) * W]
                        src = dx2[:rg, blk(b, g) * W : blk(b, g) * W + W]
                        tcol = tts[:rg, blk(b, g) * DLOC + z : blk(b, g) * DLOC + z + 1]
                        if zi == 1:
                            nc.scalar.activation(
                                out=dst, in_=src,
                                func=mybir.ActivationFunctionType.Relu,
                                bias=tcol, scale=-1.0,
                            )
                        else:
                            nc.vector.tensor_scalar(
                                out=dst, in0=src, scalar1=tcol, scalar2=None,
                                op0=op.is_le,
                            )
                    rhs.append(rhs_g)

                ps0 = psump.tile([128, NW], f32, name="ps0", tag="ps0")
                for g in range(NRG):
                    rg = rgs[g]
                    nc.tensor.matmul(
                        out=ps0[:, :],
                        lhsT=oh[:rg, blk(b, g) * W : blk(b, g) * W + 128],
                        rhs=rhs[g][:rg, :],
                        start=(g == 0), stop=(g == NRG - 1),
                    )
                for g in range(NRG):
                    rg = rgs[g]
                    nc.tensor.matmul(
                        out=ps1big[b][32 * zg : 32 * zg + 32, :],
                        lhsT=oh[:rg, blk(b, g) * W + 128 : blk(b, g) * W + 160],
                        rhs=rhs[g][:rg, :],
                        start=(g == 0), stop=(g == NRG - 1),
                        tile_position=(0, 32 * zg),
                    )

                ob0 = outp.tile([128, NW], f32, name="ob0", tag="ob0")
                if zg % 2 == 0:
                    nc.scalar.activation(
                        out=ob0[:, :], in_=ps0[:, :],
                        func=mybir.ActivationFunctionType.Sign,
                    )
                else:
                    nc.vector.tensor_scalar(
                        out=ob0[:, :], in0=ps0[:, :], scalar1=0.0, scalar2=None,
                        op0=op.is_gt,
                    )
                zl = zg * ZG
                nc.sync.dma_start(
                    out=out_d[b, 0:128, zl : zl + ZG, :],
                    in_=ob0[:, :].rearrange("p (z x) -> p z x", z=ZG),
                )

            nc.vector.tensor_scalar(
                out=ob1big[b][:, :], in0=ps1big[b][:, :], scalar1=0.0, scalar2=None,
                op0=op.is_gt,
            )
            for zg in range(NZG):
                zl = zg * ZG
                nc.sync.dma_start(
                    out=out_d[b, 128:160, zl : zl + ZG, :],
                    in_=ob1big[b][32 * zg : 32 * zg + 32, :].rearrange(
                        "p (z x) -> p z x", z=ZG
                    ),
                )

    nc.finalize()
    return nc


def _build_in_maps_tile(coords: np.ndarray, sel, NPTS):
    K = NPTS * J
    NRG = (K + 127) // 128
    NB = B * NRG
    jcol = np.tile(np.arange(-4, 6, dtype=np.float32), NPTS)

    xgridb = np.ascontiguousarray(
        np.broadcast_to(np.arange(W, dtype=np.float32), (128, W))
    )

    in_maps = []
    for core in range(NCORES):
        zbase = core * DLOC
        zgridb = np.ascontiguousarray(
            np.broadcast_to(
                np.arange(zbase, zbase + DLOC, dtype=np.float32), (128, DLOC)
            )
        )
        pts2 = np.empty((4, NB, 128), dtype=np.float32)
        pts2[0:3] = 1.0e9
        pts2[3] = 0.0
        for b in range(B):
            pb = sel[core][b]
            krows = np.repeat(pb, J, axis=0)
            n = krows.shape[0]
            for g in range(NRG):
                lo, hi = g * 128, min((g + 1) * 128, n)
                if lo >= n:
                    break
                bk = b * NRG + g
                pts2[0, bk, 0 : hi - lo] = krows[lo:hi, 0]
                pts2[1, bk, 0 : hi - lo] = krows[lo:hi, 1]
                pts2[2, bk, 0 : hi - lo] = krows[lo:hi, 2]
                pts2[3, bk, 0 : hi - lo] = jcol[lo:hi]
        in_maps.append(
            {
                "pts": np.ascontiguousarray(pts2),
                "zgridb": zgridb,
                "xgridb": xgridb,
            }
        )
    return in_maps


def _build_in_maps(coords: np.ndarray):
    sel, maxn = _select_points(coords)
    if maxn <= NPTS_FAST:
        return NPTS_FAST, _build_in_maps_fast(coords, sel)
    NPTS = ((maxn + 3) // 4) * 4
    return NPTS, _build_in_maps_tile(coords, sel, NPTS)


def _get_program(npts):
    if npts not in _prog_cache:
        if npts <= NPTS_FAST:
            _prog_cache[npts] = _build_program_fast(npts)
        else:
            _prog_cache[npts] = _build_program_tile(npts)
    return _prog_cache[npts]


def kernel(x: np.ndarray, coords: np.ndarray) -> np.ndarray:
    from concourse.bass_utils import run_bass_kernel_spmd

    assert x.shape == (B, 4, D, H, W)
    NPTS, in_maps = _build_in_maps(coords)
    nc = _get_program(NPTS)
    res = run_bass_kernel_spmd(nc, in_maps, list(range(NCORES)))

    full = np.empty((B, 1, D, H, W), dtype=np.float32)
    for core in range(NCORES):
        zsl = slice(core * DLOC, (core + 1) * DLOC)
        r = res.results[core]
        if NPTS <= NPTS_FAST:
            # out0: [4, NYC, 960] u8, pair p = b*2+h -> z in [6h,6h+6);
            # partition yc maps to y = _YACT[core][b][yc]; other y-rows are 0.
            o0 = r["out0"].reshape(B, 2, NYC, ZH, W)
            full[:, 0, zsl] = 0.0
            for b in range(B):
                yv = _YACT[core][b]
                o = o0[b, :, : len(yv)].transpose(0, 2, 1, 3)  # [2, zr, yc, x]
                # advanced indexing (b, 0 ints + yv array) puts yc first
                full[b, 0, zsl, yv, :] = o.reshape(DLOC, len(yv), W).transpose(
                    1, 0, 2
                )
        else:
            o = r["out"]  # [B, H, DLOC, W] f32
            full[:, 0, zsl] = o.transpose(0, 2, 1, 3)
    return full


# revision 23
# speedup vs baseline: 1.0467x; 1.0061x over previous
"""DistMaps kernel for Trainium2 (Bass), SPMD over 8 NeuronCores.

Problem: out[b, 0, z, y, x] = 1.0 if min_p ((z-pz)^2 + (y-py)^2 + (x-px)^2) <= 25
over the 24 points p of batch b, else 0.0.  (The x input is only used for its
shape.)

Strategy (v2)
-------------
Shard the volume over D (96 = 8 cores x 12 z-slices); each core gets only the
points whose +-5 z-window intersects its slab (<= 12, padded with far-away
dummies), so no collective is needed.

Per z-slice the plane is a union of disks; row by row a disk is an x-interval:
ind[k, x] = (x-px)^2 <= 25 - (y-py)^2 - (z-pz)^2 for k-rows (point, row offset
j in -4..5).  With onehot[k, y] = (y == floor(py)+j):

  plane[y, x] = sum_k onehot[k, y] * ind_z[k, x]   (TensorE matmul, bf16)
  out = plane > 0                                  (DVE is_gt / ACT Sign -> u8)

v2 changes vs v1 (28.9us):
  * all per-point tables (dx2[k,x], tts[k,z], onehot[k,y]) precomputed on the
    HOST and shipped as two small DMAs (~215 KB) -- removes the 4us serialized
    DVE setup chain from the critical path.
  * rhs indicators built in 4 fused DVE tensor_tensor ops (double-broadcast)
    instead of 24 sliced ops.
  * 6 matmuls instead of 16: per batch one 960-col matmul per zgroup-pair for
    y<128 and ONE 1920-col matmul for the y>=128 remainder (both batches share
    one [64,1920] PSUM via tile_position).
  * thresholds write uint8 (0/1 is exact); the host casts to f32 on unshard.
    4x fewer output bytes -> store drain ~2us instead of ~8us.
  * stores chase each pair threshold on the sync ring (overlapped with the
    rest of the pipeline).
"""

import numpy as np

B = 2
D, H, W = 96, 160, 160
P = 24
J = 10
NCORES = 8
DLOC = D // NCORES   # 12
ZG = 3               # z-slices per matmul group (tile fallback)
NZG = DLOC // ZG     # 4
R2 = 25.0
NPTS_FAST = 12
K = NPTS_FAST * J    # 120 k-rows per batch (fast path)
ZH = DLOC // 2       # 6 z-slices per zgroup-half (fast path)
NWARM = 8
WARMC = 480
NYC = 120            # compressed-y partitions (<= 120 distinct onehot rows)
USE_ACT_RHS = True

_prog_cache = {}


def _build_program_fast(npts):
    import concourse.mybir as mybir
    from concourse import bacc

    assert npts == NPTS_FAST
    f32 = mybir.dt.float32
    f16 = mybir.dt.float16
    u8 = mybir.dt.uint8
    op = mybir.AluOpType
    Act = mybir.ActivationFunctionType

    NW = ZH * W            # 960 cols per zgroup-half
    NWD = DLOC * W         # 1920 cols per batch
    HW2 = NW // 2          # 480: matmul dsts must fit in one 512-f32 PSUM bank

    nc = bacc.Bacc(trn_type="TRN2")

    # rank-space tables, fp16 (exact small ints); one chunk per batch so the
    # b0 tables are semaphore-visible earlier.
    tab0_d = nc.dram_tensor("tab0", [128, W + DLOC], f16, kind="ExternalInput")
    tab1_d = nc.dram_tensor("tab1", [128, W + 2 * DLOC], f16, kind="ExternalInput")
    # compressed onehot: oh[k, b*NYC + yc] = (yrow[k] == Y_active[b][yc])
    oh_d = nc.dram_tensor("oh", [128, 2 * NYC], f16, kind="ExternalInput")
    # out0[pair p=b*2+h, yc, (zr, x)]: z = h*6+zr, y = Y_active[b][yc]
    out0_d = nc.dram_tensor("out0", [2 * B, 128, NW], u8, kind="ExternalOutput")

    s_in = nc.alloc_semaphore("s_in")
    s_in2 = nc.alloc_semaphore("s_in2")
    s_dve = nc.alloc_semaphore("s_dve")
    s_act = nc.alloc_semaphore("s_act")
    s_pe = nc.alloc_semaphore("s_pe")
    s_st = nc.alloc_semaphore("s_st")
    sem_nums = [s.num for s in (s_in, s_in2, s_dve, s_act, s_pe, s_st)]
    assert max(sem_nums) - min(sem_nums) + 1 == len(sem_nums), sem_nums
    sem_range = range(min(sem_nums), max(sem_nums) + 1)

    tab = [
        nc.alloc_sbuf_tensor(f"tab{b}_s", [128, W + (1 + b) * DLOC], f16)
        for b in range(B)
    ]
    oh = nc.alloc_sbuf_tensor("oh_s", [128, 2 * NYC], f16)
    warm_a = nc.alloc_sbuf_tensor("warm_a", [128, 512], f16)
    rhs = [nc.alloc_sbuf_tensor(f"rhs{b}", [128, NWD], f16) for b in range(B)]
    ob0 = [nc.alloc_sbuf_tensor(f"ob0_{p}", [128, NW], u8) for p in range(2 * B)]

    # 4 ps0 slots (2 banks each) = 8 banks; no slot reuse -> no PSUM waits
    ps0 = [nc.alloc_psum_tensor(f"ps0_{i}", [128, 1024], f32) for i in range(4)]

    # ---- SP: tab0 on the sync ring ----
    nc.sync.dma_start(out=tab[0][:, :], in_=tab0_d[:, :], single_packet=True).then_inc(
        s_in, 16
    )

    # ---- SP ring: tab1 right behind tab0 (b0 tables are the critical ones) ----
    nc.sync.dma_start(out=tab[1][:, :], in_=tab1_d[:, :], single_packet=True).then_inc(
        s_in2, 16
    )

    # ---- ACT ring: onehot load, then dummy activation for the table preload ----
    nc.scalar.dma_start(out=oh[:, :], in_=oh_d[:, :], single_packet=True).then_inc(
        s_st, 16
    )
    actscratch = nc.alloc_sbuf_tensor("actscratch", [1, 4], f32)
    nc.scalar.activation(out=actscratch[0:1, :], in_=actscratch[0:1, :], func=Act.Sign)

    # ---- PE: continuous HAM warm-up until the first rhs lands ----
    for _ in range(NWARM):
        nc.tensor.matmul(
            out=ps0[0][:, 0:WARMC], lhsT=warm_a[:, 0:128], rhs=warm_a[:, 0:WARMC],
            start=True, stop=True,
        )

    dxr = [tab[b][:K, 0:W] for b in range(B)]
    ttr = [tab[b][:K, W : W + DLOC] for b in range(B)]
    c = {"dve": 0, "act": 0, "pe": 0, "st": 0}
    rhs_done = {}

    def emit_rhs(b, h, eng, sem, cnt):
        e = {"dve": nc.vector}[eng]
        e.tensor_tensor(
            out=rhs[b][:K, h * NW : (h + 1) * NW].rearrange("p (z x) -> p z x", x=W),
            in0=dxr[b].rearrange("p x -> p () x").to_broadcast([K, ZH, W]),
            in1=ttr[b][:, h * ZH : (h + 1) * ZH]
            .rearrange("p z -> p z ()")
            .to_broadcast([K, ZH, W]),
            op=op.is_le,
        ).then_inc(sem, 1)
        c[cnt] += 1
        rhs_done[(b, h)] = (cnt, c[cnt])

    nc.vector.wait_ge(s_in, 16)
    emit_rhs(0, 0, "dve", s_dve, "dve")
    emit_rhs(0, 1, "dve", s_dve, "dve")
    nc.vector.wait_ge(s_in2, 16)
    if USE_ACT_RHS:
        # ACT builds (1,0) as relu((ttr+1) - dxr): integer-exact in fp16,
        # positive iff inside, so the count threshold is unchanged.
        ttrp1 = tab[1][:K, W + DLOC : W + 2 * DLOC]
        nc.scalar.wait_ge(s_in2, 16)
        for zi in range(ZH):
            ins = nc.scalar.activation(
                out=rhs[1][:K, zi * W : (zi + 1) * W],
                in_=dxr[1],
                func=Act.Relu,
                bias=ttrp1[:, zi : zi + 1],
                scale=-1.0,
            )
        ins.then_inc(s_act, 1)
        c["act"] += 1
        rhs_done[(1, 0)] = ("act", c["act"])
    else:
        emit_rhs(1, 0, "dve", s_dve, "dve")
    emit_rhs(1, 1, "dve", s_dve, "dve")

    ohc = [oh[:K, b * NYC : (b + 1) * NYC] for b in range(B)]
    mm_done = {}
    thr_done = {}
    nc.tensor.wait_ge(s_st, 16)   # oh landed

    sems = {"dve": s_dve, "act": s_act, "in2": s_in2}

    def emit_mm_pair(b, h, slot):
        pi = b * 2 + h
        eng, v = rhs_done[(b, h)]
        nc.tensor.wait_ge(sems[eng], v)
        for q in range(2):
            ins = nc.tensor.matmul(
                out=ps0[slot][:NYC, q * 512 : q * 512 + HW2],
                lhsT=ohc[b],
                rhs=rhs[b][:K, h * NW + q * HW2 : h * NW + (q + 1) * HW2],
                start=True, stop=True,
            )
        ins.then_inc(s_pe, 1)
        c["pe"] += 1
        mm_done[pi] = c["pe"]

    def emit_thr_half(pi, q, E, slot):
        src = ps0[slot][:NYC, q * 512 : q * 512 + HW2]
        dst = ob0[pi][:NYC, q * HW2 : (q + 1) * HW2]
        if E == "dve":
            nc.vector.wait_ge(s_pe, mm_done[pi])
            nc.vector.tensor_scalar(
                out=dst, in0=src, scalar1=0.0, scalar2=None, op0=op.is_gt,
            ).then_inc(s_dve, 1)
            c["dve"] += 1
            return ("dve", c["dve"])
        else:
            nc.scalar.wait_ge(s_pe, mm_done[pi])
            nc.scalar.activation(out=dst, in_=src, func=Act.Sign).then_inc(s_act, 1)
            c["act"] += 1
            return ("act", c["act"])

    def emit_thr(pi, E, slot):
        src = ps0[slot][:NYC, :].rearrange("p (q c) -> p q c", c=512)[:, :, 0:HW2]
        dst = ob0[pi][:NYC, :].rearrange("p (q c) -> p q c", c=HW2)
        if E == "dve":
            nc.vector.wait_ge(s_pe, mm_done[pi])
            nc.vector.tensor_scalar(
                out=dst, in0=src, scalar1=0.0, scalar2=None, op0=op.is_gt,
            ).then_inc(s_dve, 1)
            c["dve"] += 1
            thr_done[pi] = [("dve", c["dve"])]
        else:
            nc.scalar.wait_ge(s_pe, mm_done[pi])
            nc.scalar.activation(out=dst, in_=src, func=Act.Sign).then_inc(s_act, 1)
            c["act"] += 1
            thr_done[pi] = [("act", c["act"])]

    emit_mm_pair(0, 0, slot=0)
    emit_mm_pair(0, 1, slot=1)
    emit_thr(0, "act", slot=0)
    emit_mm_pair(1, 0, slot=2)
    emit_thr(1, "dve", slot=1)
    emit_mm_pair(1, 1, slot=3)
    emit_thr(2, "act", slot=2)
    thr_done[3] = [
        emit_thr_half(3, 0, "dve", 3),
        emit_thr_half(3, 1, "act", 3),
    ]

    # ---- stores: st0-st2 on the sync ring, st3 on the ACT ring (its engine
    # just finished thr3's q1 half, so the config starts immediately) ----
    for pi in range(3):
        for eng, v in thr_done[pi]:
            nc.sync.wait_ge(s_dve if eng == "dve" else s_act, v)
        nc.sync.dma_start(
            out=out0_d[pi, :, :], in_=ob0[pi][:, :]
        ).then_inc(s_st, 16)
        c["st"] += 16
    (d_eng, d_v), (a_eng, a_v) = thr_done[3]
    nc.scalar.wait_ge(s_dve, d_v)
    nc.scalar.dma_start(out=out0_d[3, :, :], in_=ob0[3][:, :]).then_inc(s_st, 16)
    c["st"] += 16

    # ---- GPSIMD: wait for stores + loads, reset sems for re-execution ----
    nc.gpsimd.wait_ge(s_st, c["st"] + 16)
    nc.gpsimd.wait_ge(s_in, 16)
    nc.gpsimd.wait_ge(s_in2, 16)
    nc.gpsimd.dma_reset(sem_range)
    nc.gpsimd.sem_clear(sem_range)

    nc.finalize()
    return nc


_YACT = None  # per-core, per-batch active-y lists from the last fast build


def _build_in_maps_fast(coords: np.ndarray, sel):
    """Host-precomputed rank-space tables per core (fp16, exact small ints).

    dxr[k, x] = rank (1..160) of dx2[k, x] = (x - px)^2 among x' (stable).
    ttr[k, z] = #{x: dx2[k, x] <= tts[k, z]}  (count of in-interval x).
    Then (dx2 <= tts) <=> (dxr <= ttr) exactly.  The onehot is compressed to
    the <= 120 distinct active y-rows per (core, b); _YACT records the row
    mapping for the unshard scatter (all other y-rows are provably zero).
    """
    global _YACT
    jcol = np.arange(-4, 6, dtype=np.float32)
    xs = np.arange(W, dtype=np.float32)

    in_maps = []
    _YACT = []
    for core in range(NCORES):
        zbase = core * DLOC
        zs = np.arange(zbase, zbase + DLOC, dtype=np.float32)
        tabs = []
        oh = np.zeros((128, 2 * NYC), dtype=np.float16)
        yact_core = []
        for b in range(B):
            pb = sel[core][b]
            pts = np.full((NPTS_FAST, 3), 1.0e9, dtype=np.float32)
            pts[: len(pb)] = pb
            pz = np.repeat(pts[:, 0], J)              # (K,)
            py = np.repeat(pts[:, 1], J)
            px = np.repeat(pts[:, 2], J)
            jj = np.tile(jcol, NPTS_FAST)
            yrow = np.floor(py).astype(np.float32) + jj   # (K,)
            dyj = (yrow - py).astype(np.float32) ** 2
            cc = (np.float32(R2) - dyj).astype(np.float32)
            dz2 = ((zs[None, :] - pz[:, None]).astype(np.float32) ** 2).astype(
                np.float32
            )
            tts = (cc[:, None] - dz2).astype(np.float32)          # (K, DLOC)
            dx2 = ((xs[None, :] - px[:, None]).astype(np.float32) ** 2).astype(
                np.float32
            )                                                      # (K, W)
            order = np.argsort(dx2, axis=1, kind="stable")
            dxr = np.empty((K, W), dtype=np.float16)
            np.put_along_axis(
                dxr, order, np.arange(1, W + 1, dtype=np.float16)[None, :], axis=1
            )
            ttr = (dx2[:, None, :] <= tts[:, :, None]).sum(axis=2)  # (K, DLOC)
            tab = np.zeros((128, W + (1 + b) * DLOC), dtype=np.float16)
            tab[:K, 0:W] = dxr
            tab[:K, W : W + DLOC] = ttr.astype(np.float16)
            if b == 1:
                tab[:K, W + DLOC : W + 2 * DLOC] = (ttr + 1).astype(np.float16)
            tabs.append(np.ascontiguousarray(tab))
            # compressed onehot over the active y rows
            yint = yrow.astype(np.int64)
            yvals = np.unique(yint[(yint >= 0) & (yint < H)])
            assert len(yvals) <= NYC, len(yvals)
            yact_core.append(yvals)
            oh[:K, b * NYC : b * NYC + len(yvals)] = (
                yvals[None, :] == yrow[:, None]
            ).astype(np.float16)
        _YACT.append(yact_core)
        in_maps.append(
            {
                "tab0": tabs[0],
                "tab1": tabs[1],
                "oh": np.ascontiguousarray(oh),
            }
        )
    return in_maps


def _select_points(coords: np.ndarray):
    coords = np.ascontiguousarray(coords, dtype=np.float32)
    assert coords.shape == (B * P, 3)
    sel = []
    maxn = 0
    for core in range(NCORES):
        z0, z1 = core * DLOC, (core + 1) * DLOC
        per_b = []
        for b in range(B):
            cb = coords[b * P : (b + 1) * P]
            m = (cb[:, 0] > z0 - 5.001) & (cb[:, 0] < z1 + 5.001)
            pb = cb[m]
            per_b.append(pb)
            maxn = max(maxn, len(pb))
        sel.append(per_b)
    return sel, maxn


# ---------------------------------------------------------------------------
# Tile-scheduled fallback for NPTS > 12 (unchanged from v1).
# ---------------------------------------------------------------------------

def _build_program_tile(npts):
    from contextlib import ExitStack

    import concourse.mybir as mybir
    import concourse.tile as tile
    from concourse import bacc

    f32 = mybir.dt.float32
    bf16 = mybir.dt.bfloat16
    op = mybir.AluOpType

    K = npts * J                     # k-rows per batch
    NRG = (K + 127) // 128           # row groups per batch
    NB = B * NRG                     # total (b, g) blocks, stacked along free dim
    rgs = [min(128, K - 128 * g) for g in range(NRG)]  # rows per group

    nc = bacc.Bacc(trn_type="TRN2")

    pts_d = nc.dram_tensor("pts", [4, NB, 128], f32, kind="ExternalInput")
    zgb_d = nc.dram_tensor("zgridb", [128, DLOC], f32, kind="ExternalInput")
    xgb_d = nc.dram_tensor("xgridb", [128, W], f32, kind="ExternalInput")
    out_d = nc.dram_tensor("out", [B, H, DLOC, W], f32, kind="ExternalOutput")

    with tile.TileContext(nc) as tc, ExitStack() as ctx:
        const = ctx.enter_context(tc.tile_pool(name="const", bufs=1))
        setup = ctx.enter_context(tc.tile_pool(name="setup", bufs=1))
        rhsp = ctx.enter_context(tc.tile_pool(name="rhsp", bufs=4))
        outp = ctx.enter_context(tc.tile_pool(name="outp", bufs=8))
        psump = ctx.enter_context(tc.tile_pool(name="psump", bufs=4, space="PSUM"))
        warmp = ctx.enter_context(tc.tile_pool(name="warmp", bufs=1, space="PSUM"))

        xgb = const.tile([128, W], f32, name="xgb")
        nc.sync.dma_start(out=xgb[:, :], in_=xgb_d[:, :])
        zgb = const.tile([128, DLOC], f32, name="zgb")
        nc.sync.dma_start(out=zgb[:, :], in_=zgb_d[:, :])
        psb = setup.tile([128, 4 * NB], f32, name="psb")
        nc.sync.dma_start(out=psb[:, :], in_=pts_d[:, :])

        warm_a = const.tile([128, 512], bf16, name="warm_a")
        nc.vector.memset(warm_a[:, :], 0.0)
        warm_ps = warmp.tile([128, 512], f32, name="warm_ps")
        for w in range(14):
            nc.tensor.matmul(
                out=warm_ps[:, :], lhsT=warm_a[:, 0:128], rhs=warm_a[:, :],
                start=True, stop=True,
            )

        pzc = psb[:, 0 * NB : 1 * NB]
        pyc = psb[:, 1 * NB : 2 * NB]
        pxc = psb[:, 2 * NB : 3 * NB]
        jcc = psb[:, 3 * NB : 4 * NB]

        col = setup.tile([128, 4 * NB], f32, name="col")
        yfl = col[:, 0 * NB : 1 * NB]
        ypr = col[:, 1 * NB : 2 * NB]
        dyj = col[:, 2 * NB : 3 * NB]
        cc = col[:, 3 * NB : 4 * NB]
        MAGIC = float(12582912.0)  # 1.5 * 2**23
        nc.vector.tensor_scalar(
            out=yfl, in0=pyc, scalar1=-0.5, scalar2=MAGIC, op0=op.add, op1=op.add
        )
        nc.vector.tensor_scalar(
            out=yfl, in0=yfl, scalar1=MAGIC, scalar2=None, op0=op.subtract
        )
        nc.vector.tensor_tensor(out=ypr, in0=yfl, in1=jcc, op=op.add)
        nc.vector.tensor_tensor(out=dyj, in0=ypr, in1=pyc, op=op.subtract)
        nc.vector.tensor_tensor(out=dyj, in0=dyj, in1=dyj, op=op.mult)
        nc.vector.tensor_scalar(
            out=cc, in0=dyj, scalar1=-1.0, scalar2=R2, op0=op.mult, op1=op.add
        )

        dzt = setup.tile([128, NB * DLOC], f32, name="dzt")
        zgb_v = zgb[:, :].rearrange("p z -> p () z").to_broadcast([128, NB, DLOC])
        nc.vector.tensor_tensor(
            out=dzt[:, :].rearrange("p (blk z) -> p blk z", z=DLOC),
            in0=zgb_v,
            in1=pzc.rearrange("p blk -> p blk ()").to_broadcast([128, NB, DLOC]),
            op=op.subtract,
        )
        nc.vector.tensor_tensor(out=dzt[:, :], in0=dzt[:, :], in1=dzt[:, :], op=op.mult)
        tts = setup.tile([128, NB * DLOC], f32, name="tts")
        nc.vector.tensor_tensor(
            out=tts[:, :].rearrange("p (blk z) -> p blk z", z=DLOC),
            in0=cc.rearrange("p blk -> p blk ()").to_broadcast([128, NB, DLOC]),
            in1=dzt[:, :].rearrange("p (blk z) -> p blk z", z=DLOC),
            op=op.subtract,
        )

        dx2 = setup.tile([128, NB * W], f32, name="dx2")
        xgb_v = xgb[:, :].rearrange("p x -> p () x").to_broadcast([128, NB, W])
        nc.vector.tensor_tensor(
            out=dx2[:, :].rearrange("p (blk x) -> p blk x", x=W),
            in0=xgb_v,
            in1=pxc.rearrange("p blk -> p blk ()").to_broadcast([128, NB, W]),
            op=op.subtract,
        )
        nc.vector.tensor_tensor(out=dx2[:, :], in0=dx2[:, :], in1=dx2[:, :], op=op.mult)

        oh = setup.tile([128, NB * W], bf16, name="oh")
        nc.vector.tensor_tensor(
            out=oh[:, :].rearrange("p (blk x) -> p blk x", x=W),
            in0=xgb_v,
            in1=ypr.rearrange("p blk -> p blk ()").to_broadcast([128, NB, W]),
            op=op.is_equal,
        )

        def blk(b, g):
            return b * NRG + g

        NW = ZG * W  # 480
        ps1big = {
            b: psump.tile([128, NW], f32, name=f"ps1_{b}", tag=f"ps1_{b}", bufs=1)
            for b in range(B)
        }
        ob1big = {b: outp.tile([128, NW], f32, name=f"ob1_{b}", bufs=1) for b in range(B)}

        for b in range(B):
            for zg in range(NZG):
                rhs = []
                for g in range(NRG):
                    rg = rgs[g]
                    rhs_g = rhsp.tile([128, NW], bf16, name=f"rhs{g}", tag=f"rhs{g}")
                    for zi in range(ZG):
                        z = zg * ZG + zi
                        dst = rhs_g[:rg, zi * W : (zi + 1) * W]
                        src = dx2[:rg, blk(b, g) * W : blk(b, g) * W + W]
                        tcol = tts[:rg, blk(b, g) * DLOC + z : blk(b, g) * DLOC + z + 1]
                        if zi == 1:
                            nc.scalar.activation(
                                out=dst, in_=src,
                                func=mybir.ActivationFunctionType.Relu,
                                bias=tcol, scale=-1.0,
                            )
                        else:
                            nc.vector.tensor_scalar(
                                out=dst, in0=src, scalar1=tcol, scalar2=None,
                                op0=op.is_le,
                            )
                    rhs.append(rhs_g)

                ps0 = psump.tile([128, NW], f32, name="ps0", tag="ps0")
                for g in range(NRG):
                    rg = rgs[g]
                    nc.tensor.matmul(
                        out=ps0[:, :],
                        lhsT=oh[:rg, blk(b, g) * W : blk(b, g) * W + 128],
                        rhs=rhs[g][:rg, :],
                        start=(g == 0), stop=(g == NRG - 1),
                    )
                for g in range(NRG):
                    rg = rgs[g]
                    nc.tensor.matmul(
                        out=ps1big[b][32 * zg : 32 * zg + 32, :],
                        lhsT=oh[:rg, blk(b, g) * W + 128 : blk(b, g) * W + 160],
                        rhs=rhs[g][:rg, :],
                        start=(g == 0), stop=(g == NRG - 1),
                        tile_position=(0, 32 * zg),
                    )

                ob0 = outp.tile([128, NW], f32, name="ob0", tag="ob0")
                if zg % 2 == 0:
                    nc.scalar.activation(
                        out=ob0[:, :], in_=ps0[:, :],
                        func=mybir.ActivationFunctionType.Sign,
                    )
                else:
                    nc.vector.tensor_scalar(
                        out=ob0[:, :], in0=ps0[:, :], scalar1=0.0, scalar2=None,
                        op0=op.is_gt,
                    )
                zl = zg * ZG
                nc.sync.dma_start(
                    out=out_d[b, 0:128, zl : zl + ZG, :],
                    in_=ob0[:, :].rearrange("p (z x) -> p z x", z=ZG),
                )

            nc.vector.tensor_scalar(
                out=ob1big[b][:, :], in0=ps1big[b][:, :], scalar1=0.0, scalar2=None,
                op0=op.is_gt,
            )
            for zg in range(NZG):
                zl = zg * ZG
                nc.sync.dma_start(
                    out=out_d[b, 128:160, zl : zl + ZG, :],
                    in_=ob1big[b][32 * zg : 32 * zg + 32, :].rearrange(
                        "p (z x) -> p z x", z=ZG
                    ),
                )

    nc.finalize()
    return nc


def _build_in_maps_tile(coords: np.ndarray, sel, NPTS):
    K = NPTS * J
    NRG = (K + 127) // 128
    NB = B * NRG
    jcol = np.tile(np.arange(-4, 6, dtype=np.float32), NPTS)

    xgridb = np.ascontiguousarray(
        np.broadcast_to(np.arange(W, dtype=np.float32), (128, W))
    )

    in_maps = []
    for core in range(NCORES):
        zbase = core * DLOC
        zgridb = np.ascontiguousarray(
            np.broadcast_to(
                np.arange(zbase, zbase + DLOC, dtype=np.float32), (128, DLOC)
            )
        )
        pts2 = np.empty((4, NB, 128), dtype=np.float32)
        pts2[0:3] = 1.0e9
        pts2[3] = 0.0
        for b in range(B):
            pb = sel[core][b]
            krows = np.repeat(pb, J, axis=0)
            n = krows.shape[0]
            for g in range(NRG):
                lo, hi = g * 128, min((g + 1) * 128, n)
                if lo >= n:
                    break
                bk = b * NRG + g
                pts2[0, bk, 0 : hi - lo] = krows[lo:hi, 0]
                pts2[1, bk, 0 : hi - lo] = krows[lo:hi, 1]
                pts2[2, bk, 0 : hi - lo] = krows[lo:hi, 2]
                pts2[3, bk, 0 : hi - lo] = jcol[lo:hi]
        in_maps.append(
            {
                "pts": np.ascontiguousarray(pts2),
                "zgridb": zgridb,
                "xgridb": xgridb,
            }
        )
    return in_maps


def _build_in_maps(coords: np.ndarray):
    sel, maxn = _select_points(coords)
    if maxn <= NPTS_FAST:
        return NPTS_FAST, _build_in_maps_fast(coords, sel)
    NPTS = ((maxn + 3) // 4) * 4
    return NPTS, _build_in_maps_tile(coords, sel, NPTS)


def _get_program(npts):
    if npts not in _prog_cache:
        if npts <= NPTS_FAST:
            _prog_cache[npts] = _build_program_fast(npts)
        else:
            _prog_cache[npts] = _build_program_tile(npts)
    return _prog_cache[npts]


def kernel(x: np.ndarray, coords: np.ndarray) -> np.ndarray:
    from concourse.bass_utils import run_bass_kernel_spmd

    assert x.shape == (B, 4, D, H, W)
    NPTS, in_maps = _build_in_maps(coords)
    nc = _get_program(NPTS)
    res = run_bass_kernel_spmd(nc, in_maps, list(range(NCORES)))

    full = np.empty((B, 1, D, H, W), dtype=np.float32)
    for core in range(NCORES):
        zsl = slice(core * DLOC, (core + 1) * DLOC)
        r = res.results[core]
        if NPTS <= NPTS_FAST:
            # out0: [4, NYC, 960] u8, pair p = b*2+h -> z in [6h,6h+6);
            # partition yc maps to y = _YACT[core][b][yc]; other y-rows are 0.
            o0 = r["out0"].reshape(B, 2, 128, ZH, W)
            full[:, 0, zsl] = 0.0
            for b in range(B):
                yv = _YACT[core][b]
                o = o0[b, :, : len(yv)].transpose(0, 2, 1, 3)  # [2, zr, yc, x]
                # advanced indexing (b, 0 ints + yv array) puts yc first
                full[b, 0, zsl, yv, :] = o.reshape(DLOC, len(yv), W).transpose(
                    1, 0, 2
                )
        else:
            o = r["out"]  # [B, H, DLOC, W] f32
            full[:, 0, zsl] = o.transpose(0, 2, 1, 3)
    return full


# revision 24
# speedup vs baseline: 1.0901x; 1.0415x over previous
"""DistMaps kernel for Trainium2 (Bass), SPMD over 8 NeuronCores.

Problem: out[b, 0, z, y, x] = 1.0 if min_p ((z-pz)^2 + (y-py)^2 + (x-px)^2) <= 25
over the 24 points p of batch b, else 0.0.  (The x input is only used for its
shape.)

Strategy (v2)
-------------
Shard the volume over D (96 = 8 cores x 12 z-slices); each core gets only the
points whose +-5 z-window intersects its slab (<= 12, padded with far-away
dummies), so no collective is needed.

Per z-slice the plane is a union of disks; row by row a disk is an x-interval:
ind[k, x] = (x-px)^2 <= 25 - (y-py)^2 - (z-pz)^2 for k-rows (point, row offset
j in -4..5).  With onehot[k, y] = (y == floor(py)+j):

  plane[y, x] = sum_k onehot[k, y] * ind_z[k, x]   (TensorE matmul, bf16)
  out = plane > 0                                  (DVE is_gt / ACT Sign -> u8)

v2 changes vs v1 (28.9us):
  * all per-point tables (dx2[k,x], tts[k,z], onehot[k,y]) precomputed on the
    HOST and shipped as two small DMAs (~215 KB) -- removes the 4us serialized
    DVE setup chain from the critical path.
  * rhs indicators built in 4 fused DVE tensor_tensor ops (double-broadcast)
    instead of 24 sliced ops.
  * 6 matmuls instead of 16: per batch one 960-col matmul per zgroup-pair for
    y<128 and ONE 1920-col matmul for the y>=128 remainder (both batches share
    one [64,1920] PSUM via tile_position).
  * thresholds write uint8 (0/1 is exact); the host casts to f32 on unshard.
    4x fewer output bytes -> store drain ~2us instead of ~8us.
  * stores chase each pair threshold on the sync ring (overlapped with the
    rest of the pipeline).
"""

import numpy as np

B = 2
D, H, W = 96, 160, 160
P = 24
J = 10
NCORES = 8
DLOC = D // NCORES   # 12
ZG = 3               # z-slices per matmul group (tile fallback)
NZG = DLOC // ZG     # 4
R2 = 25.0
NPTS_FAST = 12
K = NPTS_FAST * J    # 120 k-rows per batch (fast path)
ZH = DLOC // 2       # 6 z-slices per zgroup-half (fast path)
NWARM = 8
WARMC = 480
NYC = 120            # compressed-y partitions (<= 120 distinct onehot rows)
USE_ACT_RHS = True

_prog_cache = {}


def _build_program_fast(npts):
    import concourse.mybir as mybir
    from concourse import bacc

    assert npts == NPTS_FAST
    f32 = mybir.dt.float32
    f16 = mybir.dt.float16
    u8 = mybir.dt.uint8
    op = mybir.AluOpType
    Act = mybir.ActivationFunctionType

    NW = ZH * W            # 960 cols per zgroup-half
    NWD = DLOC * W         # 1920 cols per batch
    HW2 = NW // 2          # 480: matmul dsts must fit in one 512-f32 PSUM bank

    nc = bacc.Bacc(trn_type="TRN2")

    # rank-space tables, fp16 (exact small ints); one chunk per batch so the
    # b0 tables are semaphore-visible earlier.
    tab0_d = nc.dram_tensor("tab0", [128, W + DLOC], f16, kind="ExternalInput")
    tab1_d = nc.dram_tensor("tab1", [128, W + 2 * DLOC], f16, kind="ExternalInput")
    # compressed onehot: oh[k, b*NYC + yc] = (yrow[k] == Y_active[b][yc])
    oh_d = nc.dram_tensor("oh", [128, 2 * NYC], f16, kind="ExternalInput")
    # out0[pair p=b*2+h, yc, (zr, x)]: z = h*6+zr, y = Y_active[b][yc]
    out0_d = nc.dram_tensor("out0", [128, 2 * B, NW], u8, kind="ExternalOutput")

    s_in = nc.alloc_semaphore("s_in")
    s_in2 = nc.alloc_semaphore("s_in2")
    s_dve = nc.alloc_semaphore("s_dve")
    s_act = nc.alloc_semaphore("s_act")
    s_pe = nc.alloc_semaphore("s_pe")
    s_st = nc.alloc_semaphore("s_st")
    sem_nums = [s.num for s in (s_in, s_in2, s_dve, s_act, s_pe, s_st)]
    assert max(sem_nums) - min(sem_nums) + 1 == len(sem_nums), sem_nums
    sem_range = range(min(sem_nums), max(sem_nums) + 1)

    tab = [
        nc.alloc_sbuf_tensor(f"tab{b}_s", [128, W + (1 + b) * DLOC], f16)
        for b in range(B)
    ]
    oh = nc.alloc_sbuf_tensor("oh_s", [128, 2 * NYC], f16)
    warm_a = nc.alloc_sbuf_tensor("warm_a", [128, 512], f16)
    rhs = [nc.alloc_sbuf_tensor(f"rhs{b}", [128, NWD], f16) for b in range(B)]
    ob0_all = nc.alloc_sbuf_tensor("ob0", [128, 2 * B * NW], u8)
    ob0 = [ob0_all[:, p * NW : (p + 1) * NW] for p in range(2 * B)]

    # 4 ps0 slots (2 banks each) = 8 banks; no slot reuse -> no PSUM waits
    ps0 = [nc.alloc_psum_tensor(f"ps0_{i}", [128, 1024], f32) for i in range(4)]

    # ---- SP: tab0 on the sync ring ----
    nc.sync.dma_start(out=tab[0][:, :], in_=tab0_d[:, :]).then_inc(
        s_in, 16
    )

    # ---- SP ring: tab1 right behind tab0 (b0 tables are the critical ones) ----
    nc.sync.dma_start(out=tab[1][:, :], in_=tab1_d[:, :]).then_inc(
        s_in2, 16
    )

    # ---- ACT ring: onehot load, then dummy activation for the table preload ----
    nc.scalar.dma_start(out=oh[:, :], in_=oh_d[:, :]).then_inc(
        s_st, 16
    )
    actscratch = nc.alloc_sbuf_tensor("actscratch", [1, 4], f32)
    nc.scalar.activation(out=actscratch[0:1, :], in_=actscratch[0:1, :], func=Act.Sign)

    # ---- PE: continuous HAM warm-up until the first rhs lands ----
    for _ in range(NWARM):
        nc.tensor.matmul(
            out=ps0[0][:, 0:WARMC], lhsT=warm_a[:, 0:128], rhs=warm_a[:, 0:WARMC],
            start=True, stop=True,
        )

    dxr = [tab[b][:K, 0:W] for b in range(B)]
    ttr = [tab[b][:K, W : W + DLOC] for b in range(B)]
    c = {"dve": 0, "act": 0, "pe": 0, "st": 0}
    rhs_done = {}

    def emit_rhs(b, h, eng, sem, cnt):
        e = {"dve": nc.vector}[eng]
        e.tensor_tensor(
            out=rhs[b][:K, h * NW : (h + 1) * NW].rearrange("p (z x) -> p z x", x=W),
            in0=dxr[b].rearrange("p x -> p () x").to_broadcast([K, ZH, W]),
            in1=ttr[b][:, h * ZH : (h + 1) * ZH]
            .rearrange("p z -> p z ()")
            .to_broadcast([K, ZH, W]),
            op=op.is_le,
        ).then_inc(sem, 1)
        c[cnt] += 1
        rhs_done[(b, h)] = (cnt, c[cnt])

    nc.vector.wait_ge(s_in, 16)
    emit_rhs(0, 0, "dve", s_dve, "dve")
    emit_rhs(0, 1, "dve", s_dve, "dve")
    nc.vector.wait_ge(s_in2, 16)
    if USE_ACT_RHS:
        # ACT builds (1,0) as relu((ttr+1) - dxr): integer-exact in fp16,
        # positive iff inside, so the count threshold is unchanged.
        ttrp1 = tab[1][:K, W + DLOC : W + 2 * DLOC]
        nc.scalar.wait_ge(s_in2, 16)
        for zi in range(ZH):
            ins = nc.scalar.activation(
                out=rhs[1][:K, zi * W : (zi + 1) * W],
                in_=dxr[1],
                func=Act.Relu,
                bias=ttrp1[:, zi : zi + 1],
                scale=-1.0,
            )
        ins.then_inc(s_act, 1)
        c["act"] += 1
        rhs_done[(1, 0)] = ("act", c["act"])
    else:
        emit_rhs(1, 0, "dve", s_dve, "dve")
    emit_rhs(1, 1, "dve", s_dve, "dve")

    ohc = [oh[:K, b * NYC : (b + 1) * NYC] for b in range(B)]
    mm_done = {}
    thr_done = {}
    nc.tensor.wait_ge(s_st, 16)   # oh landed

    sems = {"dve": s_dve, "act": s_act, "in2": s_in2}

    def emit_mm_pair(b, h, slot):
        pi = b * 2 + h
        eng, v = rhs_done[(b, h)]
        nc.tensor.wait_ge(sems[eng], v)
        for q in range(2):
            ins = nc.tensor.matmul(
                out=ps0[slot][:NYC, q * 512 : q * 512 + HW2],
                lhsT=ohc[b],
                rhs=rhs[b][:K, h * NW + q * HW2 : h * NW + (q + 1) * HW2],
                start=True, stop=True,
            )
        ins.then_inc(s_pe, 1)
        c["pe"] += 1
        mm_done[pi] = c["pe"]

    def emit_thr_half(pi, q, E, slot):
        src = ps0[slot][:NYC, q * 512 : q * 512 + HW2]
        dst = ob0[pi][:NYC, q * HW2 : (q + 1) * HW2]
        if E == "dve":
            nc.vector.wait_ge(s_pe, mm_done[pi])
            nc.vector.tensor_scalar(
                out=dst, in0=src, scalar1=0.0, scalar2=None, op0=op.is_gt,
            ).then_inc(s_dve, 1)
            c["dve"] += 1
            return ("dve", c["dve"])
        else:
            nc.scalar.wait_ge(s_pe, mm_done[pi])
            nc.scalar.activation(out=dst, in_=src, func=Act.Sign).then_inc(s_act, 1)
            c["act"] += 1
            return ("act", c["act"])

    def emit_thr(pi, E, slot):
        src = ps0[slot][:NYC, :].rearrange("p (q c) -> p q c", c=512)[:, :, 0:HW2]
        dst = ob0[pi][:NYC, :].rearrange("p (q c) -> p q c", c=HW2)
        if E == "dve":
            nc.vector.wait_ge(s_pe, mm_done[pi])
            nc.vector.tensor_scalar(
                out=dst, in0=src, scalar1=0.0, scalar2=None, op0=op.is_gt,
            ).then_inc(s_dve, 1)
            c["dve"] += 1
            thr_done[pi] = [("dve", c["dve"])]
        else:
            nc.scalar.wait_ge(s_pe, mm_done[pi])
            nc.scalar.activation(out=dst, in_=src, func=Act.Sign).then_inc(s_act, 1)
            c["act"] += 1
            thr_done[pi] = [("act", c["act"])]

    emit_mm_pair(0, 0, slot=0)
    emit_mm_pair(0, 1, slot=1)
    emit_thr(0, "act", slot=0)
    emit_mm_pair(1, 0, slot=2)
    emit_thr(1, "act", slot=1)
    emit_mm_pair(1, 1, slot=3)
    emit_thr(2, "dve", slot=2)
    thr_done[3] = [
        emit_thr_half(3, 0, "dve", 3),
        emit_thr_half(3, 1, "act", 3),
    ]

    # ---- stores: st0-st2 on the sync ring, st3 on the ACT ring (its engine
    # just finished thr3's q1 half, so the config starts immediately) ----
    for eng, v in thr_done[0]:
        nc.sync.wait_ge(s_dve if eng == "dve" else s_act, v)
    nc.sync.dma_start(out=out0_d[:, 0, :], in_=ob0[0][:, :]).then_inc(s_st, 16)
    c["st"] += 16
    for pi in (1, 2):
        for eng, v in thr_done[pi]:
            nc.sync.wait_ge(s_dve if eng == "dve" else s_act, v)
    nc.sync.dma_start(
        out=out0_d[:, 1:3, :], in_=ob0_all[:, NW : 3 * NW]
    ).then_inc(s_st, 16)
    c["st"] += 16
    (d_eng, d_v), (a_eng, a_v) = thr_done[3]
    nc.scalar.wait_ge(s_dve, d_v)
    nc.scalar.dma_start(out=out0_d[:, 3, :], in_=ob0[3][:, :]).then_inc(s_st, 16)
    c["st"] += 16

    # ---- GPSIMD: wait for stores + loads, reset sems for re-execution ----
    nc.gpsimd.wait_ge(s_st, c["st"] + 16)
    nc.gpsimd.wait_ge(s_in, 16)
    nc.gpsimd.wait_ge(s_in2, 16)
    nc.gpsimd.dma_reset(sem_range)
    nc.gpsimd.sem_clear(sem_range)

    nc.finalize()
    return nc


_YACT = None  # per-core, per-batch active-y lists from the last fast build


def _build_in_maps_fast(coords: np.ndarray, sel):
    """Host-precomputed rank-space tables per core (fp16, exact small ints).

    dxr[k, x] = rank (1..160) of dx2[k, x] = (x - px)^2 among x' (stable).
    ttr[k, z] = #{x: dx2[k, x] <= tts[k, z]}  (count of in-interval x).
    Then (dx2 <= tts) <=> (dxr <= ttr) exactly.  The onehot is compressed to
    the <= 120 distinct active y-rows per (core, b); _YACT records the row
    mapping for the unshard scatter (all other y-rows are provably zero).
    """
    global _YACT
    jcol = np.arange(-4, 6, dtype=np.float32)
    xs = np.arange(W, dtype=np.float32)

    in_maps = []
    _YACT = []
    for core in range(NCORES):
        zbase = core * DLOC
        zs = np.arange(zbase, zbase + DLOC, dtype=np.float32)
        tabs = []
        oh = np.zeros((128, 2 * NYC), dtype=np.float16)
        yact_core = []
        for b in range(B):
            pb = sel[core][b]
            pts = np.full((NPTS_FAST, 3), 1.0e9, dtype=np.float32)
            pts[: len(pb)] = pb
            pz = np.repeat(pts[:, 0], J)              # (K,)
            py = np.repeat(pts[:, 1], J)
            px = np.repeat(pts[:, 2], J)
            jj = np.tile(jcol, NPTS_FAST)
            yrow = np.floor(py).astype(np.float32) + jj   # (K,)
            dyj = (yrow - py).astype(np.float32) ** 2
            cc = (np.float32(R2) - dyj).astype(np.float32)
            dz2 = ((zs[None, :] - pz[:, None]).astype(np.float32) ** 2).astype(
                np.float32
            )
            tts = (cc[:, None] - dz2).astype(np.float32)          # (K, DLOC)
            dx2 = ((xs[None, :] - px[:, None]).astype(np.float32) ** 2).astype(
                np.float32
            )                                                      # (K, W)
            order = np.argsort(dx2, axis=1, kind="stable")
            dxr = np.empty((K, W), dtype=np.float16)
            np.put_along_axis(
                dxr, order, np.arange(1, W + 1, dtype=np.float16)[None, :], axis=1
            )
            ttr = (dx2[:, None, :] <= tts[:, :, None]).sum(axis=2)  # (K, DLOC)
            tab = np.zeros((128, W + (1 + b) * DLOC), dtype=np.float16)
            tab[:K, 0:W] = dxr
            tab[:K, W : W + DLOC] = ttr.astype(np.float16)
            if b == 1:
                tab[:K, W + DLOC : W + 2 * DLOC] = (ttr + 1).astype(np.float16)
            tabs.append(np.ascontiguousarray(tab))
            # compressed onehot over the active y rows
            yint = yrow.astype(np.int64)
            yvals = np.unique(yint[(yint >= 0) & (yint < H)])
            assert len(yvals) <= NYC, len(yvals)
            yact_core.append(yvals)
            oh[:K, b * NYC : b * NYC + len(yvals)] = (
                yvals[None, :] == yrow[:, None]
            ).astype(np.float16)
        _YACT.append(yact_core)
        in_maps.append(
            {
                "tab0": tabs[0],
                "tab1": tabs[1],
                "oh": np.ascontiguousarray(oh),
            }
        )
    return in_maps


def _select_points(coords: np.ndarray):
    coords = np.ascontiguousarray(coords, dtype=np.float32)
    assert coords.shape == (B * P, 3)
    sel = []
    maxn = 0
    for core in range(NCORES):
        z0, z1 = core * DLOC, (core + 1) * DLOC
        per_b = []
        for b in range(B):
            cb = coords[b * P : (b + 1) * P]
            m = (cb[:, 0] > z0 - 5.001) & (cb[:, 0] < z1 + 5.001)
            pb = cb[m]
            per_b.append(pb)
            maxn = max(maxn, len(pb))
        sel.append(per_b)
    return sel, maxn


# ---------------------------------------------------------------------------
# Tile-scheduled fallback for NPTS > 12 (unchanged from v1).
# ---------------------------------------------------------------------------

def _build_program_tile(npts):
    from contextlib import ExitStack

    import concourse.mybir as mybir
    import concourse.tile as tile
    from concourse import bacc

    f32 = mybir.dt.float32
    bf16 = mybir.dt.bfloat16
    op = mybir.AluOpType

    K = npts * J                     # k-rows per batch
    NRG = (K + 127) // 128           # row groups per batch
    NB = B * NRG                     # total (b, g) blocks, stacked along free dim
    rgs = [min(128, K - 128 * g) for g in range(NRG)]  # rows per group

    nc = bacc.Bacc(trn_type="TRN2")

    pts_d = nc.dram_tensor("pts", [4, NB, 128], f32, kind="ExternalInput")
    zgb_d = nc.dram_tensor("zgridb", [128, DLOC], f32, kind="ExternalInput")
    xgb_d = nc.dram_tensor("xgridb", [128, W], f32, kind="ExternalInput")
    out_d = nc.dram_tensor("out", [B, H, DLOC, W], f32, kind="ExternalOutput")

    with tile.TileContext(nc) as tc, ExitStack() as ctx:
        const = ctx.enter_context(tc.tile_pool(name="const", bufs=1))
        setup = ctx.enter_context(tc.tile_pool(name="setup", bufs=1))
        rhsp = ctx.enter_context(tc.tile_pool(name="rhsp", bufs=4))
        outp = ctx.enter_context(tc.tile_pool(name="outp", bufs=8))
        psump = ctx.enter_context(tc.tile_pool(name="psump", bufs=4, space="PSUM"))
        warmp = ctx.enter_context(tc.tile_pool(name="warmp", bufs=1, space="PSUM"))

        xgb = const.tile([128, W], f32, name="xgb")
        nc.sync.dma_start(out=xgb[:, :], in_=xgb_d[:, :])
        zgb = const.tile([128, DLOC], f32, name="zgb")
        nc.sync.dma_start(out=zgb[:, :], in_=zgb_d[:, :])
        psb = setup.tile([128, 4 * NB], f32, name="psb")
        nc.sync.dma_start(out=psb[:, :], in_=pts_d[:, :])

        warm_a = const.tile([128, 512], bf16, name="warm_a")
        nc.vector.memset(warm_a[:, :], 0.0)
        warm_ps = warmp.tile([128, 512], f32, name="warm_ps")
        for w in range(14):
            nc.tensor.matmul(
                out=warm_ps[:, :], lhsT=warm_a[:, 0:128], rhs=warm_a[:, :],
                start=True, stop=True,
            )

        pzc = psb[:, 0 * NB : 1 * NB]
        pyc = psb[:, 1 * NB : 2 * NB]
        pxc = psb[:, 2 * NB : 3 * NB]
        jcc = psb[:, 3 * NB : 4 * NB]

        col = setup.tile([128, 4 * NB], f32, name="col")
        yfl = col[:, 0 * NB : 1 * NB]
        ypr = col[:, 1 * NB : 2 * NB]
        dyj = col[:, 2 * NB : 3 * NB]
        cc = col[:, 3 * NB : 4 * NB]
        MAGIC = float(12582912.0)  # 1.5 * 2**23
        nc.vector.tensor_scalar(
            out=yfl, in0=pyc, scalar1=-0.5, scalar2=MAGIC, op0=op.add, op1=op.add
        )
        nc.vector.tensor_scalar(
            out=yfl, in0=yfl, scalar1=MAGIC, scalar2=None, op0=op.subtract
        )
        nc.vector.tensor_tensor(out=ypr, in0=yfl, in1=jcc, op=op.add)
        nc.vector.tensor_tensor(out=dyj, in0=ypr, in1=pyc, op=op.subtract)
        nc.vector.tensor_tensor(out=dyj, in0=dyj, in1=dyj, op=op.mult)
        nc.vector.tensor_scalar(
            out=cc, in0=dyj, scalar1=-1.0, scalar2=R2, op0=op.mult, op1=op.add
        )

        dzt = setup.tile([128, NB * DLOC], f32, name="dzt")
        zgb_v = zgb[:, :].rearrange("p z -> p () z").to_broadcast([128, NB, DLOC])
        nc.vector.tensor_tensor(
            out=dzt[:, :].rearrange("p (blk z) -> p blk z", z=DLOC),
            in0=zgb_v,
            in1=pzc.rearrange("p blk -> p blk ()").to_broadcast([128, NB, DLOC]),
            op=op.subtract,
        )
        nc.vector.tensor_tensor(out=dzt[:, :], in0=dzt[:, :], in1=dzt[:, :], op=op.mult)
        tts = setup.tile([128, NB * DLOC], f32, name="tts")
        nc.vector.tensor_tensor(
            out=tts[:, :].rearrange("p (blk z) -> p blk z", z=DLOC),
            in0=cc.rearrange("p blk -> p blk ()").to_broadcast([128, NB, DLOC]),
            in1=dzt[:, :].rearrange("p (blk z) -> p blk z", z=DLOC),
            op=op.subtract,
        )

        dx2 = setup.tile([128, NB * W], f32, name="dx2")
        xgb_v = xgb[:, :].rearrange("p x -> p () x").to_broadcast([128, NB, W])
        nc.vector.tensor_tensor(
            out=dx2[:, :].rearrange("p (blk x) -> p blk x", x=W),
            in0=xgb_v,
            in1=pxc.rearrange("p blk -> p blk ()").to_broadcast([128, NB, W]),
            op=op.subtract,
        )
        nc.vector.tensor_tensor(out=dx2[:, :], in0=dx2[:, :], in1=dx2[:, :], op=op.mult)

        oh = setup.tile([128, NB * W], bf16, name="oh")
        nc.vector.tensor_tensor(
            out=oh[:, :].rearrange("p (blk x) -> p blk x", x=W),
            in0=xgb_v,
            in1=ypr.rearrange("p blk -> p blk ()").to_broadcast([128, NB, W]),
            op=op.is_equal,
        )

        def blk(b, g):
            return b * NRG + g

        NW = ZG * W  # 480
        ps1big = {
            b: psump.tile([128, NW], f32, name=f"ps1_{b}", tag=f"ps1_{b}", bufs=1)
            for b in range(B)
        }
        ob1big = {b: outp.tile([128, NW], f32, name=f"ob1_{b}", bufs=1) for b in range(B)}

        for b in range(B):
            for zg in range(NZG):
                rhs = []
                for g in range(NRG):
                    rg = rgs[g]
                    rhs_g = rhsp.tile([128, NW], bf16, name=f"rhs{g}", tag=f"rhs{g}")
                    for zi in range(ZG):
                        z = zg * ZG + zi
                        dst = rhs_g[:rg, zi * W : (zi + 1) * W]
                        src = dx2[:rg, blk(b, g) * W : blk(b, g) * W + W]
                        tcol = tts[:rg, blk(b, g) * DLOC + z : blk(b, g) * DLOC + z + 1]
                        if zi == 1:
                            nc.scalar.activation(
                                out=dst, in_=src,
                                func=mybir.ActivationFunctionType.Relu,
                                bias=tcol, scale=-1.0,
                            )
                        else:
                            nc.vector.tensor_scalar(
                                out=dst, in0=src, scalar1=tcol, scalar2=None,
                                op0=op.is_le,
                            )
                    rhs.append(rhs_g)

                ps0 = psump.tile([128, NW], f32, name="ps0", tag="ps0")
                for g in range(NRG):
                    rg = rgs[g]
                    nc.tensor.matmul(
                        out=ps0[:, :],
                        lhsT=oh[:rg, blk(b, g) * W : blk(b, g) * W + 128],
                        rhs=rhs[g][:rg, :],
                        start=(g == 0), stop=(g == NRG - 1),
                    )
                for g in range(NRG):
                    rg = rgs[g]
                    nc.tensor.matmul(
                        out=ps1big[b][32 * zg : 32 * zg + 32, :],
                        lhsT=oh[:rg, blk(b, g) * W + 128 : blk(b, g) * W + 160],
                        rhs=rhs[g][:rg, :],
                        start=(g == 0), stop=(g == NRG - 1),
                        tile_position=(0, 32 * zg),
                    )

                ob0 = outp.tile([128, NW], f32, name="ob0", tag="ob0")
                if zg % 2 == 0:
                    nc.scalar.activation(
                        out=ob0[:, :], in_=ps0[:, :],
                        func=mybir.ActivationFunctionType.Sign,
                    )
                else:
                    nc.vector.tensor_scalar(
                        out=ob0[:, :], in0=ps0[:, :], scalar1=0.0, scalar2=None,
                        op0=op.is_gt,
                    )
                zl = zg * ZG
                nc.sync.dma_start(
                    out=out_d[b, 0:128, zl : zl + ZG, :],
                    in_=ob0[:, :].rearrange("p (z x) -> p z x", z=ZG),
                )

            nc.vector.tensor_scalar(
                out=ob1big[b][:, :], in0=ps1big[b][:, :], scalar1=0.0, scalar2=None,
                op0=op.is_gt,
            )
            for zg in range(NZG):
                zl = zg * ZG
                nc.sync.dma_start(
                    out=out_d[b, 128:160, zl : zl + ZG, :],
                    in_=ob1big[b][32 * zg : 32 * zg + 32, :].rearrange(
                        "p (z x) -> p z x", z=ZG
                    ),
                )

    nc.finalize()
    return nc


def _build_in_maps_tile(coords: np.ndarray, sel, NPTS):
    K = NPTS * J
    NRG = (K + 127) // 128
    NB = B * NRG
    jcol = np.tile(np.arange(-4, 6, dtype=np.float32), NPTS)

    xgridb = np.ascontiguousarray(
        np.broadcast_to(np.arange(W, dtype=np.float32), (128, W))
    )

    in_maps = []
    for core in range(NCORES):
        zbase = core * DLOC
        zgridb = np.ascontiguousarray(
            np.broadcast_to(
                np.arange(zbase, zbase + DLOC, dtype=np.float32), (128, DLOC)
            )
        )
        pts2 = np.empty((4, NB, 128), dtype=np.float32)
        pts2[0:3] = 1.0e9
        pts2[3] = 0.0
        for b in range(B):
            pb = sel[core][b]
            krows = np.repeat(pb, J, axis=0)
            n = krows.shape[0]
            for g in range(NRG):
                lo, hi = g * 128, min((g + 1) * 128, n)
                if lo >= n:
                    break
                bk = b * NRG + g
                pts2[0, bk, 0 : hi - lo] = krows[lo:hi, 0]
                pts2[1, bk, 0 : hi - lo] = krows[lo:hi, 1]
                pts2[2, bk, 0 : hi - lo] = krows[lo:hi, 2]
                pts2[3, bk, 0 : hi - lo] = jcol[lo:hi]
        in_maps.append(
            {
                "pts": np.ascontiguousarray(pts2),
                "zgridb": zgridb,
                "xgridb": xgridb,
            }
        )
    return in_maps


def _build_in_maps(coords: np.ndarray):
    sel, maxn = _select_points(coords)
    if maxn <= NPTS_FAST:
        return NPTS_FAST, _build_in_maps_fast(coords, sel)
    NPTS = ((maxn + 3) // 4) * 4
    return NPTS, _build_in_maps_tile(coords, sel, NPTS)


def _get_program(npts):
    if npts not in _prog_cache:
        if npts <= NPTS_FAST:
            _prog_cache[npts] = _build_program_fast(npts)
        else:
            _prog_cache[npts] = _build_program_tile(npts)
    return _prog_cache[npts]


def kernel(x: np.ndarray, coords: np.ndarray) -> np.ndarray:
    from concourse.bass_utils import run_bass_kernel_spmd

    assert x.shape == (B, 4, D, H, W)
    NPTS, in_maps = _build_in_maps(coords)
    nc = _get_program(NPTS)
    res = run_bass_kernel_spmd(nc, in_maps, list(range(NCORES)))

    full = np.empty((B, 1, D, H, W), dtype=np.float32)
    for core in range(NCORES):
        zsl = slice(core * DLOC, (core + 1) * DLOC)
        r = res.results[core]
        if NPTS <= NPTS_FAST:
            # out0: [4, NYC, 960] u8, pair p = b*2+h -> z in [6h,6h+6);
            # partition yc maps to y = _YACT[core][b][yc]; other y-rows are 0.
            o0 = (
                r["out0"].reshape(128, B, 2, ZH, W).transpose(1, 2, 0, 3, 4)
            )
            full[:, 0, zsl] = 0.0
            for b in range(B):
                yv = _YACT[core][b]
                o = o0[b, :, : len(yv)].transpose(0, 2, 1, 3)  # [2, zr, yc, x]
                # advanced indexing (b, 0 ints + yv array) puts yc first
                full[b, 0, zsl, yv, :] = o.reshape(DLOC, len(yv), W).transpose(
                    1, 0, 2
                )
        else:
            o = r["out"]  # [B, H, DLOC, W] f32
            full[:, 0, zsl] = o.transpose(0, 2, 1, 3)
    return full
